# revision 9
# baseline (speedup 1.0000x reference)
"""MimicAcquisition (double resample: nearest-at-acquisition-res then trilinear
back) as three separable banded-matrix contractions on the PE engine, in bf16,
with *used-row compaction*.

out[i,j,k] = sum_{a,b,c} Ax[a,i] * Ay[b,j] * Az[c,k] * vol[a,b,c]

A_d = (trilinear upsample) @ (nearest resample) along axis d.  The nearest
resample for subsample_res r reads only ~192/r distinct source rows, so A_d
has few nonzero rows.  Host gathers exactly those rows: the per-core slab
shrinks from 112x112x192 to <=80 x <=36 x <=88 (seed-0 inputs), cutting input
DMA ~9.5x and all matmul contractions proportionally.

Each of the 8 cores handles one (batch, i-half, j-half) octant.  Device:

  pass Y: t2[z, j, x]   = sum_y slab[y, x, z] * Ay[y, j]      (per-x weights)
  pass Z: t3[(j2,x), k] = sum_z t2[z, j-pair, x] * Az[z, k]   (128-col FWL)
  pass X: out[i, j, k]  = sum_x Ax[x, i] * t3[x-slice, k]     (row+col tiled)

t2 packs j-pairs at a 64-element x stride so pass-Z weights are a contiguous
128-column block and the psZ partition layout (j0 @ 0-63, j1 @ 64-127) feeds
pass X with 32-aligned rhs partition bases.  Pass X splits i into [0,64) and
[64,96) blocks at four array tile positions, so psum tiles carry 128 useful
partitions and the two row strips run concurrently.  PSUM evacuation (the
bottleneck: DVE/ACT are the only PSUM readers, 1 elem/cycle/partition) is
greedily balanced across vector+scalar by a build-time cost model.
"""

import sys

if "/opt/trn_rl_repo" not in sys.path:
    sys.path.insert(0, "/opt/trn_rl_repo")

import numpy as np

IN = 192          # input extent per axis
RES = 192         # resample (output) extent per axis
OH = 96           # output half extent for sharded axes (i, j)
NYC = 80          # compacted y rows (max used: 77)
NXC = 36          # compacted x rows per i-half (max used: 35)
NZC = 88          # compacted z rows, full axis (max used: 85)
XP = 64           # x stride in t2: 2 j per 128-column weight block
NPAIR = OH // 2   # 48 j-pairs
NQUAD = OH // 4   # 24 j-quads

_CACHE = {}

LAST_RESULTS = None


# ----------------------------------------------------------------------------
# Host-side table construction (mirrors reference.py float32 arithmetic)
# ----------------------------------------------------------------------------

def _axis_matrix(r):
    """A[src, dst] for one axis given subsample resolution r (float32)."""
    f32 = np.float32
    d = (f32(IN) * f32(1.0) / f32(r)).astype(np.int32)  # down_shape (trunc)
    dz = f32(d) / f32(IN)                               # down_zoom
    uz = f32(RES) / f32(d)                              # up_zoom
    maxl = f32(IN - 1)

    # pass 2 (trilinear) locations for output index i, in mid coordinates
    i = np.arange(RES, dtype=np.float32)
    loc = np.clip(i / uz, f32(0.0), maxl)
    loc0 = np.floor(loc)
    f0 = np.clip(loc0, f32(0.0), maxl)
    f1 = np.clip(loc0 + f32(1.0), f32(0.0), maxl)
    w0 = (f1 - loc).astype(np.float32)      # weight for floor corner
    w1 = (f32(1.0) - w0).astype(np.float32)
    i0 = f0.astype(np.int32)
    i1 = f1.astype(np.int32)

    # pass 1 (nearest) map applied to mid index j
    j = np.arange(IN, dtype=np.float32)
    dl = np.clip(j / dz, f32(0.0), f32(IN))
    g = np.clip(np.round(dl), f32(0.0), maxl).astype(np.int32)

    A = np.zeros((IN, RES), dtype=np.float32)
    cols = np.arange(RES)
    A[g[i0], cols] += w0
    A[g[i1], cols] += w1
    return A


def _used_rows(A, lo, n):
    """Indices of rows of A with any nonzero in dst columns [lo, lo+n)."""
    return np.nonzero(np.any(A[:, lo:lo + n] != 0.0, axis=1))[0]


# ----------------------------------------------------------------------------
# Device kernel (built once per process)
# ----------------------------------------------------------------------------

def _build(bench_iters=0):
    key = ("nc", bench_iters)
    if key in _CACHE:
        return _CACHE[key]

    import contextlib

    import concourse.mybir as mybir
    from concourse import bacc, tile

    bf16 = mybir.dt.bfloat16
    f32 = mybir.dt.float32
    nc = bacc.Bacc("TRN2", debug=False)

    slab_d = nc.dram_tensor("slab", (NYC, NXC, NZC), bf16, kind="ExternalInput")
    ay_d = nc.dram_tensor("ay", (NYC, OH), bf16, kind="ExternalInput")
    az_d = nc.dram_tensor("az", (NZC, RES), bf16, kind="ExternalInput")
    # Full-K pass-X weights: axe rows [0, NXC) hold Ax (even j), axo rows
    # [64, 64+NXC) (odd j); all other rows zero.  K=128 matmuls against the
    # full t3 block then pipeline at full-array cadence (no row tiling).
    axe_d = nc.dram_tensor("axe", (128, OH), bf16, kind="ExternalInput")
    axo_d = nc.dram_tensor("axo", (128, OH), bf16, kind="ExternalInput")
    # out columns are j-permuted per 2-quad block: c=8u+4qq+2*eo+h maps to
    # j=4*(2u+qq)+2*h+eo (host unpermutes)
    out_d = nc.dram_tensor("out", (OH, OH, RES), bf16, kind="ExternalOutput")

    CHUNKS = [4, 8, 12, 12]     # slab x-chunk DMA sizes (small first chunk
    assert sum(CHUNKS) == NXC   # so pass Y starts early)
    XG = 4                      # x per pass-Y psum sub-slot
    NU = NQUAD // 2             # 2-quad super-groups (12)

    # build-time greedy balance of psum-evac copies across vector/scalar,
    # using HW-measured per-op costs (PSUM-read bubble included)
    eng_ns = {"v": 0.0, "s": 0.0}

    def evac(nc_, dst, src, n_free):
        if eng_ns["v"] + (n_free + 250) / 0.96 <= eng_ns["s"] + (n_free + 450) / 1.2:
            eng_ns["v"] += (n_free + 250) / 0.96
            nc_.vector.tensor_copy(dst, src)
        else:
            eng_ns["s"] += (n_free + 450) / 1.2
            nc_.scalar.copy(dst, src)

    with tile.TileContext(nc) as tc:
        loop_cm = (
            tc.For_i(0, bench_iters, 1) if bench_iters
            else contextlib.nullcontext()
        )
        with (
            loop_cm,
            tc.tile_pool(name="consts", bufs=1) as consts,
            tc.tile_pool(name="slab", bufs=1) as slab_pool,
            tc.tile_pool(name="t2", bufs=1) as t2_pool,
            tc.tile_pool(name="t3", bufs=3) as t3_pool,
            tc.tile_pool(name="stg", bufs=3) as stg_pool,
        ):
            ay_t = consts.tile([NYC, OH], bf16, tag="ay")
            az_t = consts.tile([NZC, RES], bf16, tag="az")
            axe_t = consts.tile([128, OH], bf16, tag="axe")
            axo_t = consts.tile([128, OH], bf16, tag="axo")

            slabs = []
            x0s = []
            x0 = 0
            for ci, cw in enumerate(CHUNKS):
                s = slab_pool.tile([NYC, cw, NZC], bf16, tag=f"s{ci}")
                slabs.append(s)
                x0s.append(x0)
                x0 += cw
            # chunk 0 and ay gate the first matmul: issue them first on
            # separate queues so they land in parallel
            nc.sync.dma_start(slabs[0][:], slab_d[:, 0:CHUNKS[0], :])
            nc.scalar.dma_start(ay_t[:], ay_d[:])
            nc.sync.dma_start(slabs[1][:], slab_d[:, x0s[1]:x0s[1] + CHUNKS[1], :])
            nc.scalar.dma_start(slabs[2][:], slab_d[:, x0s[2]:x0s[2] + CHUNKS[2], :])
            nc.sync.dma_start(slabs[3][:], slab_d[:, x0s[3]:x0s[3] + CHUNKS[3], :])
            nc.scalar.dma_start(az_t[:], az_d[:])
            nc.sync.dma_start(axe_t[:], axe_d[:])
            nc.scalar.dma_start(axo_t[:], axo_d[:])

            # t2[z, j, x] with 64-wide x: pad cols [NXC, XP) zeroed once so
            # pass-Z weight blocks (and hence t3 pad rows) are exact zeros
            t2 = t2_pool.tile([NZC, OH, XP], bf16, tag="t2")
            nc.gpsimd.memset(t2[:, :, NXC:XP], 0.0)

            def chunk_of(x):
                for ci, lo in enumerate(x0s):
                    if lo <= x < lo + CHUNKS[ci]:
                        return ci, x - lo
                raise AssertionError(x)

            # ---- pass Y: t2[z, j, x] = sum_y slab[y, x, z] * Ay[y, j] ----
            # psum supers span 2 banks (8 x-slices; bank-aligned 512B slots)
            with tc.tile_pool(name="psY", bufs=2, space="PSUM") as psy_pool:
                for xg in range(NXC // (2 * XG)):
                    ps = psy_pool.tile([NZC, 2 * XG, 128], f32, tag="psY8")
                    for xi in range(2 * XG):
                        x = xg * 2 * XG + xi
                        ci, xl = chunk_of(x)
                        nc.tensor.matmul(
                            ps[:, xi, 0:OH], slabs[ci][:, xl, :], ay_t[:]
                        )
                    dst = t2[:, :, xg * 2 * XG:(xg + 1) * 2 * XG]
                    evac(nc, dst, ps[:, :, 0:OH].transpose([0, 2, 1]), 2 * XG * OH)
                # ragged tail (4 slices)
                ps = psy_pool.tile([NZC, XG, 128], f32, tag="psY4")
                for xi in range(XG):
                    x = (NXC // (2 * XG)) * 2 * XG + xi
                    ci, xl = chunk_of(x)
                    nc.tensor.matmul(ps[:, xi, 0:OH], slabs[ci][:, xl, :], ay_t[:])
                dst = t2[:, :, NXC - XG:NXC]
                evac(nc, dst, ps[:, :, 0:OH].transpose([0, 2, 1]), XG * OH)

            # ---- pass Z + pass X, per 2-quad super-group (8 j) ----
            # software-pipelined one group deep: Z-MMs of group u issue
            # before X-MMs of group u-1, so an X stall (psum-evac wait)
            # never blocks the independent Z stream behind it
            with (
                tc.tile_pool(name="psZ", bufs=2, space="PSUM") as psz_pool,
                tc.tile_pool(name="psX", bufs=2, space="PSUM") as psx_pool,
            ):
                dma_flip = [0]

                def emit_x(t3, u):
                    stg = stg_pool.tile([OH, 8, RES], bf16, tag="stg")
                    for qq in range(2):
                        # pass X: full-K matmuls over the quad's t3 block
                        px = psx_pool.tile([OH, 2, 512], f32, tag="px")
                        rhs = t3[:, 2 * qq:2 * qq + 2, :]
                        nc.tensor.matmul(px[:, 0, 0:2 * RES], axe_t[:], rhs)
                        nc.tensor.matmul(px[:, 1, 0:2 * RES], axo_t[:], rhs)
                        dst = stg[:, 4 * qq:4 * qq + 4, :]
                        evac(nc, dst, px[:, :, 0:2 * RES], 4 * RES)
                        # store per quad (permuted j; host unshuffles)
                        eng = nc.sync if dma_flip[0] % 2 == 0 else nc.gpsimd
                        dma_flip[0] += 1
                        eng.dma_start(
                            out_d[:, 8 * u + 4 * qq:8 * u + 4 * qq + 4, :],
                            stg[:, 4 * qq:4 * qq + 4, :],
                        )

                prev = None
                for u in range(NU):
                    # pass Z: 4 pairs -> psZ super [128, 4, 192] (2 banks)
                    pz = psz_pool.tile([128, 4, 256], f32, tag="pz")
                    for s in range(4):
                        p = 4 * u + s
                        nc.tensor.matmul(
                            pz[:, s, 0:RES], t2[:, 2 * p:2 * p + 2, :], az_t[:]
                        )
                    t3 = t3_pool.tile([128, 4, RES], bf16, tag="t3")
                    evac(nc, t3[:], pz[:, :, 0:RES], 4 * RES)
                    if prev is not None:
                        emit_x(*prev)
                    prev = (t3, u)
                emit_x(*prev)

    nc.compile()
    _CACHE[key] = nc
    return nc


# ----------------------------------------------------------------------------
# Host wrapper
# ----------------------------------------------------------------------------

def _in_maps(vol, sub):
    """Compact per-core inputs; returns None if any compacted dim overflows."""
    import ml_dtypes

    bf16 = ml_dtypes.bfloat16
    maps = []
    spans = []
    tabs = {}
    for core in range(8):
        b = core >> 2
        ix = (core >> 1) & 1
        iy = core & 1
        if b not in tabs:
            tabs[b] = tuple(_axis_matrix(sub[b, d]) for d in range(3))
        Ax, Ay, Az = tabs[b]
        xr = _used_rows(Ax, ix * OH, OH)
        yr = _used_rows(Ay, iy * OH, OH)
        zr = _used_rows(Az, 0, RES)
        if len(xr) > NXC or len(yr) > NYC or len(zr) > NZC:
            return None, None
        axc = Ax[xr][:, ix * OH:(ix + 1) * OH]
        axe = np.zeros((128, OH), dtype=np.float32)
        axe[0:len(xr)] = axc
        axo = np.zeros((128, OH), dtype=np.float32)
        axo[64:64 + len(xr)] = axc
        ayp = np.zeros((NYC, OH), dtype=np.float32)
        ayp[0:len(yr)] = Ay[yr][:, iy * OH:(iy + 1) * OH]
        azp = np.zeros((NZC, RES), dtype=np.float32)
        azp[0:len(zr)] = Az[zr]
        slab = np.zeros((NYC, NXC, NZC), dtype=np.float32)
        slab[0:len(yr), 0:len(xr), 0:len(zr)] = (
            vol[b, :, :, :, 0][np.ix_(xr, yr, zr)].transpose(1, 0, 2)
        )
        maps.append({
            "slab": slab.astype(bf16),
            "axe": axe.astype(bf16),
            "axo": axo.astype(bf16),
            "ay": ayp.astype(bf16),
            "az": azp.astype(bf16),
        })
        spans.append((b, ix * OH, iy * OH))
    return maps, spans


def kernel(vol, subsample_res):
    global LAST_RESULTS
    from concourse import bass_utils

    vol = np.asarray(vol, dtype=np.float32)
    sub = np.asarray(subsample_res, dtype=np.float32)
    maps, spans = _in_maps(vol, sub)
    if maps is None:
        return _kernel_fallback(vol, sub)
    nc = _build()
    res = bass_utils.run_bass_kernel_spmd(nc, maps, core_ids=list(range(8)))
    LAST_RESULTS = res
    # out column c = 8u + 4qq + 2*eo + h holds j = 4*(2u+qq) + 2*h + eo
    jmap = np.empty(OH, dtype=np.int64)
    for q in range(NQUAD):
        for eo in range(2):
            for h in range(2):
                jmap[4 * q + 2 * eo + h] = 4 * q + 2 * h + eo
    out = np.empty((2, RES, RES, RES, 1), dtype=np.float32)
    for core, (b, x0, y0) in enumerate(spans):
        blk = np.asarray(res.results[core]["out"], dtype=np.float32)
        out[b, x0:x0 + OH, y0 + jmap, :, 0] = blk.transpose(1, 0, 2)
    return out


# ----------------------------------------------------------------------------
# Fallback: original banded-slab kernel (handles any subsample_res in [1, 4])
# ----------------------------------------------------------------------------

H = 112           # padded source-band rows for the sharded axes (x, y)
XPF = H
Z = 192
C0 = 122
Z1LO = 116
Z1N = Z - Z1LO


def _band_slice(A, lo, n):
    cols = A[:, lo:lo + n]
    rows = np.nonzero(np.any(cols != 0.0, axis=1))[0]
    rmin, rmax = int(rows[0]), int(rows[-1])
    assert rmax - rmin + 1 <= H, f"band too wide: {rmax - rmin + 1}"
    S0 = min(rmin, IN - H)
    assert rmax < S0 + H
    return S0, np.ascontiguousarray(cols[S0:S0 + H])


def _build_fb(bench_iters=0):
    key = ("nc_fb", bench_iters)
    if key in _CACHE:
        return _CACHE[key]

    import contextlib

    import concourse.mybir as mybir
    from concourse import bacc, tile

    bf16 = mybir.dt.bfloat16
    nc = bacc.Bacc("TRN2", debug=False)

    slab_d = nc.dram_tensor("slab", (H, H, Z), bf16, kind="ExternalInput")
    ax_d = nc.dram_tensor("ax", (XPF, OH), bf16, kind="ExternalInput")
    ay_d = nc.dram_tensor("ay", (H, OH), bf16, kind="ExternalInput")
    az0_d = nc.dram_tensor("az0", (128, C0), bf16, kind="ExternalInput")
    az1_d = nc.dram_tensor("az1", (Z1N, RES - C0), bf16, kind="ExternalInput")
    out_d = nc.dram_tensor("out", (OH, OH, Z), bf16, kind="ExternalOutput")

    CHUNKS = [4, 8, 12, 24, 32, 32]
    assert sum(CHUNKS) == H
    XG = 4
    JB = 8
    NB = OH // JB

    with tile.TileContext(nc) as tc:
        loop_cm = (
            tc.For_i(0, bench_iters, 1) if bench_iters
            else contextlib.nullcontext()
        )
        with (
            loop_cm,
            tc.tile_pool(name="consts", bufs=1) as consts,
            tc.tile_pool(name="slab", bufs=1) as slab_pool,
            tc.tile_pool(name="t2", bufs=1) as t2_pool,
            tc.tile_pool(name="t3", bufs=2) as t3_pool,
            tc.tile_pool(name="stage", bufs=3) as stage_pool,
        ):
            ay_t = consts.tile([H, OH], bf16, tag="ay")
            az0_t = consts.tile([128, C0], bf16, tag="az0")
            az1_t = consts.tile([Z1N, RES - C0], bf16, tag="az1")
            ax_t = consts.tile([XPF, OH], bf16, tag="ax")

            slabs = []
            x0s = []
            x0 = 0
            for ci, cw in enumerate(CHUNKS):
                s = slab_pool.tile([H, cw, Z], bf16, tag=f"s{ci}")
                slabs.append(s)
                x0s.append(x0)
                x0 += cw
            nc.sync.dma_start(slabs[0][:], slab_d[:, 0:CHUNKS[0], :])
            nc.scalar.dma_start(slabs[1][:], slab_d[:, x0s[1]:x0s[1] + CHUNKS[1], :])
            nc.sync.dma_start(ay_t[:], ay_d[:])
            nc.scalar.dma_start(az0_t[:], az0_d[:])
            nc.scalar.dma_start(az1_t[:], az1_d[:])
            nc.scalar.dma_start(ax_t[:], ax_d[:])
            for ci in range(2, len(CHUNKS)):
                eng = nc.sync if ci % 2 == 0 else nc.scalar
                eng.dma_start(
                    slabs[ci][:], slab_d[:, x0s[ci]:x0s[ci] + CHUNKS[ci], :]
                )

            t2a = t2_pool.tile([128, OH, XPF], bf16, tag="t2a")
            t2b = t2_pool.tile([Z1N, OH, XPF], bf16, tag="t2b")

            def chunk_of(x):
                for ci, lo in enumerate(x0s):
                    if lo <= x < lo + CHUNKS[ci]:
                        return ci, x - lo
                raise AssertionError(x)

            with (
                tc.tile_pool(name="psumw", bufs=1, space="PSUM") as psumw,
                tc.tile_pool(name="psum1", bufs=3, space="PSUM") as psum1,
            ):
                warm = consts.tile([1, 512], bf16, tag="warm")
                nc.gpsimd.memset(warm[:], 0.0)
                psw = psumw.tile([128, 512], mybir.dt.float32, tag="psw")
                for _ in range(30):
                    nc.tensor.matmul(psw[:], warm[:, 0:128], warm[:])

                for xg in range(H // XG):
                    psA = psum1.tile([128, XG, OH], mybir.dt.float32, tag="psA")
                    psB = psum1.tile([Z1N, XG, OH], mybir.dt.float32, tag="psB")
                    for xi in range(XG):
                        x = xg * XG + xi
                        ci, xl = chunk_of(x)
                        s = slabs[ci]
                        nc.tensor.matmul(psA[:, xi, :], s[:, xl, 0:128], ay_t[:])
                        nc.tensor.matmul(psB[:, xi, :], s[:, xl, Z1LO:Z], ay_t[:])
                    lo = xg * XG
                    dstA = t2a[:, :, lo:lo + XG]
                    dstB = t2b[:, :, lo:lo + XG]
                    srcA = psA[:].transpose([0, 2, 1])
                    srcB = psB[:].transpose([0, 2, 1])
                    if xg % 2 == 0:
                        nc.scalar.copy(dstA, srcA)
                        nc.vector.tensor_copy(dstB, srcB)
                    else:
                        nc.vector.tensor_copy(dstA, srcA)
                        nc.scalar.copy(dstB, srcB)

            with tc.tile_pool(name="psum2", bufs=3, space="PSUM") as psum2:
                for jb in range(NB):
                    t3 = t3_pool.tile([XPF, JB, Z], bf16, tag="t3")
                    stage = stage_pool.tile([OH, JB, Z], bf16, tag="st")
                    for jg in range(JB // 2):
                        pz = psum2.tile([XPF, 2, RES], mybir.dt.float32, tag="pz")
                        for ji in range(2):
                            j = jb * JB + jg * 2 + ji
                            nc.tensor.matmul(pz[:, ji, 0:C0], t2a[:, j, :], az0_t[:])
                            nc.tensor.matmul(pz[:, ji, C0:RES], t2b[:, j, :], az1_t[:])
                        t3d = t3[:, jg * 2:jg * 2 + 2, :]
                        px = psum2.tile([OH, 2, Z], mybir.dt.float32, tag="px")
                        std = stage[:, jg * 2:jg * 2 + 2, :]
                        if jg % 2 == 0:
                            nc.scalar.copy(t3d, pz[:])
                            nc.tensor.matmul(px[:], ax_t[:], t3d)
                            nc.vector.tensor_copy(std, px[:])
                        else:
                            nc.vector.tensor_copy(t3d, pz[:])
                            nc.tensor.matmul(px[:], ax_t[:], t3d)
                            nc.scalar.copy(std, px[:])
                    j0 = jb * JB
                    if jb == NB - 1:
                        qj = JB // 4
                        for q in range(4):
                            eng = nc.gpsimd if q % 2 == 0 else nc.sync
                            eng.dma_start(
                                out_d[:, j0 + q * qj:j0 + (q + 1) * qj, :],
                                stage[:, q * qj:(q + 1) * qj, :],
                            )
                    elif jb % 2 == 0:
                        nc.gpsimd.dma_start(out_d[:, j0:j0 + JB, :], stage[:])
                    else:
                        nc.sync.dma_start(out_d[:, j0:j0 + JB, :], stage[:])

    nc.compile()
    _CACHE[key] = nc
    return nc


def _in_maps_fb(vol, sub):
    import ml_dtypes

    bf16 = ml_dtypes.bfloat16
    maps = []
    spans = []
    tabs = {}
    for core in range(8):
        b = core >> 2
        ix = (core >> 1) & 1
        iy = core & 1
        if b not in tabs:
            tabs[b] = tuple(_axis_matrix(sub[b, d]) for d in range(3))
        Ax, Ay, Az = tabs[b]
        SX, axs = _band_slice(Ax, ix * OH, OH)
        SY, ays = _band_slice(Ay, iy * OH, OH)
        axp = np.zeros((XPF, OH), dtype=np.float32)
        axp[:H] = axs
        assert not Az[128:, :C0].any(), "az0 band bound violated"
        assert not Az[:Z1LO, C0:].any(), "az1 band bound violated"
        az0 = np.ascontiguousarray(Az[0:128, 0:C0])
        az1 = np.ascontiguousarray(Az[Z1LO:Z, C0:RES])
        slab = np.ascontiguousarray(
            vol[b, SX:SX + H, SY:SY + H, :, 0].transpose(1, 0, 2)
        )
        maps.append({
            "slab": slab.astype(bf16),
            "ax": axp.astype(bf16),
            "ay": ays.astype(bf16),
            "az0": az0.astype(bf16),
            "az1": az1.astype(bf16),
        })
        spans.append((b, ix * OH, iy * OH))
    return maps, spans


def _kernel_fallback(vol, sub):
    global LAST_RESULTS
    from concourse import bass_utils

    nc = _build_fb()
    maps, spans = _in_maps_fb(vol, sub)
    res = bass_utils.run_bass_kernel_spmd(nc, maps, core_ids=list(range(8)))
    LAST_RESULTS = res
    out = np.empty((2, RES, RES, RES, 1), dtype=np.float32)
    for core, (b, x0, y0) in enumerate(spans):
        out[b, x0:x0 + OH, y0:y0 + OH, :, 0] = np.asarray(
            res.results[core]["out"], dtype=np.float32
        )
    return out


# revision 10
# speedup vs baseline: 1.0440x; 1.0440x over previous
"""MimicAcquisition (double resample: nearest-at-acquisition-res then trilinear
back) as three separable banded-matrix contractions on the PE engine, in bf16,
with *used-row compaction*.

out[i,j,k] = sum_{a,b,c} Ax[a,i] * Ay[b,j] * Az[c,k] * vol[a,b,c]

A_d = (trilinear upsample) @ (nearest resample) along axis d.  The nearest
resample for subsample_res r reads only ~192/r distinct source rows, so A_d
has few nonzero rows.  Host gathers exactly those rows: the per-core slab
shrinks from 112x112x192 to <=80 x <=36 x <=88 (seed-0 inputs), cutting input
DMA ~9.5x and all matmul contractions proportionally.

Each of the 8 cores handles one (batch, i-half, j-half) octant.  Device:

  pass Y: t2[z, j, x]   = sum_y slab[y, x, z] * Ay[y, j]      (per-x weights)
  pass Z: t3[(j2,x), k] = sum_z t2[z, j-pair, x] * Az[z, k]   (128-col FWL)
  pass X: out[i, j, k]  = sum_x Ax[x, i] * t3[x-slice, k]     (row+col tiled)

t2 packs j-pairs at a 64-element x stride so pass-Z weights are a contiguous
128-column block and the psZ partition layout (j0 @ 0-63, j1 @ 64-127) feeds
pass X with 32-aligned rhs partition bases.  Pass X splits i into [0,64) and
[64,96) blocks at four array tile positions, so psum tiles carry 128 useful
partitions and the two row strips run concurrently.  PSUM evacuation (the
bottleneck: DVE/ACT are the only PSUM readers, 1 elem/cycle/partition) is
greedily balanced across vector+scalar by a build-time cost model.
"""

import sys

if "/opt/trn_rl_repo" not in sys.path:
    sys.path.insert(0, "/opt/trn_rl_repo")

import numpy as np

IN = 192          # input extent per axis
RES = 192         # resample (output) extent per axis
OH = 96           # output half extent for sharded axes (i, j)
NYC = 80          # compacted y rows (max used: 77)
NXC = 36          # compacted x rows per i-half (max used: 35)
NZC = 88          # compacted z rows, full axis (max used: 85)
XP = 64           # x stride in t2: 2 j per 128-column weight block
NPAIR = OH // 2   # 48 j-pairs
NQUAD = OH // 4   # 24 j-quads

_CACHE = {}

LAST_RESULTS = None


# ----------------------------------------------------------------------------
# Host-side table construction (mirrors reference.py float32 arithmetic)
# ----------------------------------------------------------------------------

def _axis_matrix(r):
    """A[src, dst] for one axis given subsample resolution r (float32)."""
    f32 = np.float32
    d = (f32(IN) * f32(1.0) / f32(r)).astype(np.int32)  # down_shape (trunc)
    dz = f32(d) / f32(IN)                               # down_zoom
    uz = f32(RES) / f32(d)                              # up_zoom
    maxl = f32(IN - 1)

    # pass 2 (trilinear) locations for output index i, in mid coordinates
    i = np.arange(RES, dtype=np.float32)
    loc = np.clip(i / uz, f32(0.0), maxl)
    loc0 = np.floor(loc)
    f0 = np.clip(loc0, f32(0.0), maxl)
    f1 = np.clip(loc0 + f32(1.0), f32(0.0), maxl)
    w0 = (f1 - loc).astype(np.float32)      # weight for floor corner
    w1 = (f32(1.0) - w0).astype(np.float32)
    i0 = f0.astype(np.int32)
    i1 = f1.astype(np.int32)

    # pass 1 (nearest) map applied to mid index j
    j = np.arange(IN, dtype=np.float32)
    dl = np.clip(j / dz, f32(0.0), f32(IN))
    g = np.clip(np.round(dl), f32(0.0), maxl).astype(np.int32)

    A = np.zeros((IN, RES), dtype=np.float32)
    cols = np.arange(RES)
    A[g[i0], cols] += w0
    A[g[i1], cols] += w1
    return A


def _used_rows(A, lo, n):
    """Indices of rows of A with any nonzero in dst columns [lo, lo+n)."""
    return np.nonzero(np.any(A[:, lo:lo + n] != 0.0, axis=1))[0]


# ----------------------------------------------------------------------------
# Device kernel (built once per process)
# ----------------------------------------------------------------------------

def _build(bench_iters=0):
    key = ("nc", bench_iters)
    if key in _CACHE:
        return _CACHE[key]

    import contextlib

    import concourse.mybir as mybir
    from concourse import bacc, tile

    bf16 = mybir.dt.bfloat16
    f32 = mybir.dt.float32
    nc = bacc.Bacc("TRN2", debug=False)

    slab_d = nc.dram_tensor("slab", (NYC, NXC, NZC), bf16, kind="ExternalInput")
    ay_d = nc.dram_tensor("ay", (NYC, OH), bf16, kind="ExternalInput")
    az_d = nc.dram_tensor("az", (NZC, RES), bf16, kind="ExternalInput")
    # Full-K pass-X weights: axe rows [0, NXC) hold Ax (even j), axo rows
    # [64, 64+NXC) (odd j); all other rows zero.  K=128 matmuls against the
    # full t3 block then pipeline at full-array cadence (no row tiling).
    axe_d = nc.dram_tensor("axe", (128, OH), bf16, kind="ExternalInput")
    axo_d = nc.dram_tensor("axo", (128, OH), bf16, kind="ExternalInput")
    # out columns are j-permuted per 2-quad block: c=8u+4qq+2*eo+h maps to
    # j=4*(2u+qq)+2*h+eo (host unpermutes)
    out_d = nc.dram_tensor("out", (OH, OH, RES), bf16, kind="ExternalOutput")

    CHUNKS = [4, 8, 12, 12]     # slab x-chunk DMA sizes (small first chunk
    assert sum(CHUNKS) == NXC   # so pass Y starts early)
    XG = 4                      # x per pass-Y psum sub-slot
    NU = NQUAD // 2             # 2-quad super-groups (12)

    # build-time greedy balance of psum-evac copies across vector/scalar,
    # using HW-measured per-op costs (PSUM-read bubble included)
    eng_ns = {"v": 0.0, "s": 0.0}

    def evac(nc_, dst, src, n_free):
        if eng_ns["v"] + (n_free + 250) / 0.96 <= eng_ns["s"] + (n_free + 450) / 1.2:
            eng_ns["v"] += (n_free + 250) / 0.96
            nc_.vector.tensor_copy(dst, src)
        else:
            eng_ns["s"] += (n_free + 450) / 1.2
            nc_.scalar.copy(dst, src)

    with tile.TileContext(nc) as tc:
        loop_cm = (
            tc.For_i(0, bench_iters, 1) if bench_iters
            else contextlib.nullcontext()
        )
        with (
            loop_cm,
            tc.tile_pool(name="consts", bufs=1) as consts,
            tc.tile_pool(name="slab", bufs=1) as slab_pool,
            tc.tile_pool(name="t2", bufs=1) as t2_pool,
            tc.tile_pool(name="t3", bufs=3) as t3_pool,
            tc.tile_pool(name="stg", bufs=6) as stg_pool,
        ):
            ay_t = consts.tile([NYC, OH], bf16, tag="ay")
            az_t = consts.tile([NZC, RES], bf16, tag="az")
            axe_t = consts.tile([128, OH], bf16, tag="axe")
            axo_t = consts.tile([128, OH], bf16, tag="axo")

            slabs = []
            x0s = []
            x0 = 0
            for ci, cw in enumerate(CHUNKS):
                s = slab_pool.tile([NYC, cw, NZC], bf16, tag=f"s{ci}")
                slabs.append(s)
                x0s.append(x0)
                x0 += cw
            # chunk 0 and ay gate the first matmul: issue them first on
            # separate queues so they land in parallel
            nc.sync.dma_start(slabs[0][:], slab_d[:, 0:CHUNKS[0], :])
            nc.scalar.dma_start(ay_t[:], ay_d[:])
            nc.sync.dma_start(slabs[1][:], slab_d[:, x0s[1]:x0s[1] + CHUNKS[1], :])
            nc.scalar.dma_start(slabs[2][:], slab_d[:, x0s[2]:x0s[2] + CHUNKS[2], :])
            nc.sync.dma_start(slabs[3][:], slab_d[:, x0s[3]:x0s[3] + CHUNKS[3], :])
            nc.scalar.dma_start(az_t[:], az_d[:])
            nc.sync.dma_start(axe_t[:], axe_d[:])
            nc.scalar.dma_start(axo_t[:], axo_d[:])

            # t2[z, j, x] with 64-wide x: pad cols [NXC, XP) zeroed once so
            # pass-Z weight blocks (and hence t3 pad rows) are exact zeros
            t2 = t2_pool.tile([NZC, OH, XP], bf16, tag="t2")
            nc.gpsimd.memset(t2[:, :, NXC:XP], 0.0)

            def chunk_of(x):
                for ci, lo in enumerate(x0s):
                    if lo <= x < lo + CHUNKS[ci]:
                        return ci, x - lo
                raise AssertionError(x)

            # ---- pass Y: t2[z, j, x] = sum_y slab[y, x, z] * Ay[y, j] ----
            # psum supers span 2 banks (8 x-slices; bank-aligned 512B slots)
            with tc.tile_pool(name="psY", bufs=2, space="PSUM") as psy_pool:
                for xg in range(NXC // (2 * XG)):
                    ps = psy_pool.tile([NZC, 2 * XG, 128], f32, tag="psY8")
                    for xi in range(2 * XG):
                        x = xg * 2 * XG + xi
                        ci, xl = chunk_of(x)
                        nc.tensor.matmul(
                            ps[:, xi, 0:OH], slabs[ci][:, xl, :], ay_t[:]
                        )
                    dst = t2[:, :, xg * 2 * XG:(xg + 1) * 2 * XG]
                    evac(nc, dst, ps[:, :, 0:OH].transpose([0, 2, 1]), 2 * XG * OH)
                # ragged tail (4 slices)
                ps = psy_pool.tile([NZC, XG, 128], f32, tag="psY4")
                for xi in range(XG):
                    x = (NXC // (2 * XG)) * 2 * XG + xi
                    ci, xl = chunk_of(x)
                    nc.tensor.matmul(ps[:, xi, 0:OH], slabs[ci][:, xl, :], ay_t[:])
                dst = t2[:, :, NXC - XG:NXC]
                evac(nc, dst, ps[:, :, 0:OH].transpose([0, 2, 1]), XG * OH)

            # ---- pass Z + pass X, per 2-quad super-group (8 j) ----
            # software-pipelined one group deep: Z-MMs of group u issue
            # before X-MMs of group u-1, so an X stall (psum-evac wait)
            # never blocks the independent Z stream behind it
            with (
                tc.tile_pool(name="psZ", bufs=2, space="PSUM") as psz_pool,
                tc.tile_pool(name="psX", bufs=2, space="PSUM") as psx_pool,
            ):
                dma_flip = [0]

                def emit_x(t3, u):
                    stg = stg_pool.tile([OH, 8, RES], bf16, tag="stg")
                    for qq in range(2):
                        # pass X: full-K matmuls over the quad's t3 block
                        px = psx_pool.tile([OH, 2, 512], f32, tag="px")
                        rhs = t3[:, 2 * qq:2 * qq + 2, :]
                        nc.tensor.matmul(px[:, 0, 0:2 * RES], axe_t[:], rhs)
                        nc.tensor.matmul(px[:, 1, 0:2 * RES], axo_t[:], rhs)
                        dst = stg[:, 4 * qq:4 * qq + 4, :]
                        evac(nc, dst, px[:, :, 0:2 * RES], 4 * RES)
                        # store per quad (permuted j; host unshuffles)
                        eng = nc.sync if dma_flip[0] % 2 == 0 else nc.gpsimd
                        dma_flip[0] += 1
                        eng.dma_start(
                            out_d[:, 8 * u + 4 * qq:8 * u + 4 * qq + 4, :],
                            stg[:, 4 * qq:4 * qq + 4, :],
                        )

                prev = None
                for u in range(NU):
                    # pass Z: 4 pairs -> psZ super [128, 4, 192] (2 banks)
                    pz = psz_pool.tile([128, 4, 256], f32, tag="pz")
                    for s in range(4):
                        p = 4 * u + s
                        nc.tensor.matmul(
                            pz[:, s, 0:RES], t2[:, 2 * p:2 * p + 2, :], az_t[:]
                        )
                    t3 = t3_pool.tile([128, 4, RES], bf16, tag="t3")
                    evac(nc, t3[:], pz[:, :, 0:RES], 4 * RES)
                    if prev is not None:
                        emit_x(*prev)
                    prev = (t3, u)
                emit_x(*prev)

    nc.compile()
    _CACHE[key] = nc
    return nc


# ----------------------------------------------------------------------------
# Host wrapper
# ----------------------------------------------------------------------------

def _in_maps(vol, sub):
    """Compact per-core inputs; returns None if any compacted dim overflows."""
    import ml_dtypes

    bf16 = ml_dtypes.bfloat16
    maps = []
    spans = []
    tabs = {}
    for core in range(8):
        b = core >> 2
        ix = (core >> 1) & 1
        iy = core & 1
        if b not in tabs:
            tabs[b] = tuple(_axis_matrix(sub[b, d]) for d in range(3))
        Ax, Ay, Az = tabs[b]
        xr = _used_rows(Ax, ix * OH, OH)
        yr = _used_rows(Ay, iy * OH, OH)
        zr = _used_rows(Az, 0, RES)
        if len(xr) > NXC or len(yr) > NYC or len(zr) > NZC:
            return None, None
        axc = Ax[xr][:, ix * OH:(ix + 1) * OH]
        axe = np.zeros((128, OH), dtype=np.float32)
        axe[0:len(xr)] = axc
        axo = np.zeros((128, OH), dtype=np.float32)
        axo[64:64 + len(xr)] = axc
        ayp = np.zeros((NYC, OH), dtype=np.float32)
        ayp[0:len(yr)] = Ay[yr][:, iy * OH:(iy + 1) * OH]
        azp = np.zeros((NZC, RES), dtype=np.float32)
        azp[0:len(zr)] = Az[zr]
        slab = np.zeros((NYC, NXC, NZC), dtype=np.float32)
        slab[0:len(yr), 0:len(xr), 0:len(zr)] = (
            vol[b, :, :, :, 0][np.ix_(xr, yr, zr)].transpose(1, 0, 2)
        )
        maps.append({
            "slab": slab.astype(bf16),
            "axe": axe.astype(bf16),
            "axo": axo.astype(bf16),
            "ay": ayp.astype(bf16),
            "az": azp.astype(bf16),
        })
        spans.append((b, ix * OH, iy * OH))
    return maps, spans


def kernel(vol, subsample_res):
    global LAST_RESULTS
    from concourse import bass_utils

    vol = np.asarray(vol, dtype=np.float32)
    sub = np.asarray(subsample_res, dtype=np.float32)
    maps, spans = _in_maps(vol, sub)
    if maps is None:
        return _kernel_fallback(vol, sub)
    nc = _build()
    res = bass_utils.run_bass_kernel_spmd(nc, maps, core_ids=list(range(8)))
    LAST_RESULTS = res
    # out column c = 8u + 4qq + 2*eo + h holds j = 4*(2u+qq) + 2*h + eo
    jmap = np.empty(OH, dtype=np.int64)
    for q in range(NQUAD):
        for eo in range(2):
            for h in range(2):
                jmap[4 * q + 2 * eo + h] = 4 * q + 2 * h + eo
    out = np.empty((2, RES, RES, RES, 1), dtype=np.float32)
    for core, (b, x0, y0) in enumerate(spans):
        blk = np.asarray(res.results[core]["out"], dtype=np.float32)
        out[b, x0:x0 + OH, y0 + jmap, :, 0] = blk.transpose(1, 0, 2)
    return out


# ----------------------------------------------------------------------------
# Fallback: original banded-slab kernel (handles any subsample_res in [1, 4])
# ----------------------------------------------------------------------------

H = 112           # padded source-band rows for the sharded axes (x, y)
XPF = H
Z = 192
C0 = 122
Z1LO = 116
Z1N = Z - Z1LO


def _band_slice(A, lo, n):
    cols = A[:, lo:lo + n]
    rows = np.nonzero(np.any(cols != 0.0, axis=1))[0]
    rmin, rmax = int(rows[0]), int(rows[-1])
    assert rmax - rmin + 1 <= H, f"band too wide: {rmax - rmin + 1}"
    S0 = min(rmin, IN - H)
    assert rmax < S0 + H
    return S0, np.ascontiguousarray(cols[S0:S0 + H])


def _build_fb(bench_iters=0):
    key = ("nc_fb", bench_iters)
    if key in _CACHE:
        return _CACHE[key]

    import contextlib

    import concourse.mybir as mybir
    from concourse import bacc, tile

    bf16 = mybir.dt.bfloat16
    nc = bacc.Bacc("TRN2", debug=False)

    slab_d = nc.dram_tensor("slab", (H, H, Z), bf16, kind="ExternalInput")
    ax_d = nc.dram_tensor("ax", (XPF, OH), bf16, kind="ExternalInput")
    ay_d = nc.dram_tensor("ay", (H, OH), bf16, kind="ExternalInput")
    az0_d = nc.dram_tensor("az0", (128, C0), bf16, kind="ExternalInput")
    az1_d = nc.dram_tensor("az1", (Z1N, RES - C0), bf16, kind="ExternalInput")
    out_d = nc.dram_tensor("out", (OH, OH, Z), bf16, kind="ExternalOutput")

    CHUNKS = [4, 8, 12, 24, 32, 32]
    assert sum(CHUNKS) == H
    XG = 4
    JB = 8
    NB = OH // JB

    with tile.TileContext(nc) as tc:
        loop_cm = (
            tc.For_i(0, bench_iters, 1) if bench_iters
            else contextlib.nullcontext()
        )
        with (
            loop_cm,
            tc.tile_pool(name="consts", bufs=1) as consts,
            tc.tile_pool(name="slab", bufs=1) as slab_pool,
            tc.tile_pool(name="t2", bufs=1) as t2_pool,
            tc.tile_pool(name="t3", bufs=2) as t3_pool,
            tc.tile_pool(name="stage", bufs=3) as stage_pool,
        ):
            ay_t = consts.tile([H, OH], bf16, tag="ay")
            az0_t = consts.tile([128, C0], bf16, tag="az0")
            az1_t = consts.tile([Z1N, RES - C0], bf16, tag="az1")
            ax_t = consts.tile([XPF, OH], bf16, tag="ax")

            slabs = []
            x0s = []
            x0 = 0
            for ci, cw in enumerate(CHUNKS):
                s = slab_pool.tile([H, cw, Z], bf16, tag=f"s{ci}")
                slabs.append(s)
                x0s.append(x0)
                x0 += cw
            nc.sync.dma_start(slabs[0][:], slab_d[:, 0:CHUNKS[0], :])
            nc.scalar.dma_start(slabs[1][:], slab_d[:, x0s[1]:x0s[1] + CHUNKS[1], :])
            nc.sync.dma_start(ay_t[:], ay_d[:])
            nc.scalar.dma_start(az0_t[:], az0_d[:])
            nc.scalar.dma_start(az1_t[:], az1_d[:])
            nc.scalar.dma_start(ax_t[:], ax_d[:])
            for ci in range(2, len(CHUNKS)):
                eng = nc.sync if ci % 2 == 0 else nc.scalar
                eng.dma_start(
                    slabs[ci][:], slab_d[:, x0s[ci]:x0s[ci] + CHUNKS[ci], :]
                )

            t2a = t2_pool.tile([128, OH, XPF], bf16, tag="t2a")
            t2b = t2_pool.tile([Z1N, OH, XPF], bf16, tag="t2b")

            def chunk_of(x):
                for ci, lo in enumerate(x0s):
                    if lo <= x < lo + CHUNKS[ci]:
                        return ci, x - lo
                raise AssertionError(x)

            with (
                tc.tile_pool(name="psumw", bufs=1, space="PSUM") as psumw,
                tc.tile_pool(name="psum1", bufs=3, space="PSUM") as psum1,
            ):
                warm = consts.tile([1, 512], bf16, tag="warm")
                nc.gpsimd.memset(warm[:], 0.0)
                psw = psumw.tile([128, 512], mybir.dt.float32, tag="psw")
                for _ in range(30):
                    nc.tensor.matmul(psw[:], warm[:, 0:128], warm[:])

                for xg in range(H // XG):
                    psA = psum1.tile([128, XG, OH], mybir.dt.float32, tag="psA")
                    psB = psum1.tile([Z1N, XG, OH], mybir.dt.float32, tag="psB")
                    for xi in range(XG):
                        x = xg * XG + xi
                        ci, xl = chunk_of(x)
                        s = slabs[ci]
                        nc.tensor.matmul(psA[:, xi, :], s[:, xl, 0:128], ay_t[:])
                        nc.tensor.matmul(psB[:, xi, :], s[:, xl, Z1LO:Z], ay_t[:])
                    lo = xg * XG
                    dstA = t2a[:, :, lo:lo + XG]
                    dstB = t2b[:, :, lo:lo + XG]
                    srcA = psA[:].transpose([0, 2, 1])
                    srcB = psB[:].transpose([0, 2, 1])
                    if xg % 2 == 0:
                        nc.scalar.copy(dstA, srcA)
                        nc.vector.tensor_copy(dstB, srcB)
                    else:
                        nc.vector.tensor_copy(dstA, srcA)
                        nc.scalar.copy(dstB, srcB)

            with tc.tile_pool(name="psum2", bufs=3, space="PSUM") as psum2:
                for jb in range(NB):
                    t3 = t3_pool.tile([XPF, JB, Z], bf16, tag="t3")
                    stage = stage_pool.tile([OH, JB, Z], bf16, tag="st")
                    for jg in range(JB // 2):
                        pz = psum2.tile([XPF, 2, RES], mybir.dt.float32, tag="pz")
                        for ji in range(2):
                            j = jb * JB + jg * 2 + ji
                            nc.tensor.matmul(pz[:, ji, 0:C0], t2a[:, j, :], az0_t[:])
                            nc.tensor.matmul(pz[:, ji, C0:RES], t2b[:, j, :], az1_t[:])
                        t3d = t3[:, jg * 2:jg * 2 + 2, :]
                        px = psum2.tile([OH, 2, Z], mybir.dt.float32, tag="px")
                        std = stage[:, jg * 2:jg * 2 + 2, :]
                        if jg % 2 == 0:
                            nc.scalar.copy(t3d, pz[:])
                            nc.tensor.matmul(px[:], ax_t[:], t3d)
                            nc.vector.tensor_copy(std, px[:])
                        else:
                            nc.vector.tensor_copy(t3d, pz[:])
                            nc.tensor.matmul(px[:], ax_t[:], t3d)
                            nc.scalar.copy(std, px[:])
                    j0 = jb * JB
                    if jb == NB - 1:
                        qj = JB // 4
                        for q in range(4):
                            eng = nc.gpsimd if q % 2 == 0 else nc.sync
                            eng.dma_start(
                                out_d[:, j0 + q * qj:j0 + (q + 1) * qj, :],
                                stage[:, q * qj:(q + 1) * qj, :],
                            )
                    elif jb % 2 == 0:
                        nc.gpsimd.dma_start(out_d[:, j0:j0 + JB, :], stage[:])
                    else:
                        nc.sync.dma_start(out_d[:, j0:j0 + JB, :], stage[:])

    nc.compile()
    _CACHE[key] = nc
    return nc


def _in_maps_fb(vol, sub):
    import ml_dtypes

    bf16 = ml_dtypes.bfloat16
    maps = []
    spans = []
    tabs = {}
    for core in range(8):
        b = core >> 2
        ix = (core >> 1) & 1
        iy = core & 1
        if b not in tabs:
            tabs[b] = tuple(_axis_matrix(sub[b, d]) for d in range(3))
        Ax, Ay, Az = tabs[b]
        SX, axs = _band_slice(Ax, ix * OH, OH)
        SY, ays = _band_slice(Ay, iy * OH, OH)
        axp = np.zeros((XPF, OH), dtype=np.float32)
        axp[:H] = axs
        assert not Az[128:, :C0].any(), "az0 band bound violated"
        assert not Az[:Z1LO, C0:].any(), "az1 band bound violated"
        az0 = np.ascontiguousarray(Az[0:128, 0:C0])
        az1 = np.ascontiguousarray(Az[Z1LO:Z, C0:RES])
        slab = np.ascontiguousarray(
            vol[b, SX:SX + H, SY:SY + H, :, 0].transpose(1, 0, 2)
        )
        maps.append({
            "slab": slab.astype(bf16),
            "ax": axp.astype(bf16),
            "ay": ays.astype(bf16),
            "az0": az0.astype(bf16),
            "az1": az1.astype(bf16),
        })
        spans.append((b, ix * OH, iy * OH))
    return maps, spans


def _kernel_fallback(vol, sub):
    global LAST_RESULTS
    from concourse import bass_utils

    nc = _build_fb()
    maps, spans = _in_maps_fb(vol, sub)
    res = bass_utils.run_bass_kernel_spmd(nc, maps, core_ids=list(range(8)))
    LAST_RESULTS = res
    out = np.empty((2, RES, RES, RES, 1), dtype=np.float32)
    for core, (b, x0, y0) in enumerate(spans):
        out[b, x0:x0 + OH, y0:y0 + OH, :, 0] = np.asarray(
            res.results[core]["out"], dtype=np.float32
        )
    return out


# revision 14
# speedup vs baseline: 1.0513x; 1.0070x over previous
"""MimicAcquisition (double resample: nearest-at-acquisition-res then trilinear
back) as three separable banded-matrix contractions on the PE engine, in bf16,
with *used-row compaction*.

out[i,j,k] = sum_{a,b,c} Ax[a,i] * Ay[b,j] * Az[c,k] * vol[a,b,c]

A_d = (trilinear upsample) @ (nearest resample) along axis d.  The nearest
resample for subsample_res r reads only ~192/r distinct source rows, so A_d
has few nonzero rows.  Host gathers exactly those rows: the per-core slab
shrinks from 112x112x192 to <=80 x <=36 x <=88 (seed-0 inputs), cutting input
DMA ~9.5x and all matmul contractions proportionally.

Each of the 8 cores handles one (batch, i-half, j-half) octant.  Device:

  pass Y: t2[z, j, x]   = sum_y slab[y, x, z] * Ay[y, j]      (per-x weights)
  pass Z: t3[(j2,x), k] = sum_z t2[z, j-pair, x] * Az[z, k]   (128-col FWL)
  pass X: out[i, j, k]  = sum_x Ax[x, i] * t3[x-slice, k]     (row+col tiled)

t2 packs j-pairs at a 64-element x stride so pass-Z weights are a contiguous
128-column block and the psZ partition layout (j0 @ 0-63, j1 @ 64-127) feeds
pass X with 32-aligned rhs partition bases.  Pass X splits i into [0,64) and
[64,96) blocks at four array tile positions, so psum tiles carry 128 useful
partitions and the two row strips run concurrently.  PSUM evacuation (the
bottleneck: DVE/ACT are the only PSUM readers, 1 elem/cycle/partition) is
greedily balanced across vector+scalar by a build-time cost model.
"""

import sys

if "/opt/trn_rl_repo" not in sys.path:
    sys.path.insert(0, "/opt/trn_rl_repo")

import numpy as np

IN = 192          # input extent per axis
RES = 192         # resample (output) extent per axis
OH = 96           # output half extent for sharded axes (i, j)
NYC = 80          # compacted y rows (max used: 77)
NXC = 36          # compacted x rows per i-half (max used: 35)
NZC = 88          # compacted z rows, full axis (max used: 85)
XP = 64           # x stride in t2: 2 j per 128-column weight block
NPAIR = OH // 2   # 48 j-pairs
NQUAD = OH // 4   # 24 j-quads

_CACHE = {}

LAST_RESULTS = None


# ----------------------------------------------------------------------------
# Host-side table construction (mirrors reference.py float32 arithmetic)
# ----------------------------------------------------------------------------

def _axis_matrix(r):
    """A[src, dst] for one axis given subsample resolution r (float32)."""
    f32 = np.float32
    d = (f32(IN) * f32(1.0) / f32(r)).astype(np.int32)  # down_shape (trunc)
    dz = f32(d) / f32(IN)                               # down_zoom
    uz = f32(RES) / f32(d)                              # up_zoom
    maxl = f32(IN - 1)

    # pass 2 (trilinear) locations for output index i, in mid coordinates
    i = np.arange(RES, dtype=np.float32)
    loc = np.clip(i / uz, f32(0.0), maxl)
    loc0 = np.floor(loc)
    f0 = np.clip(loc0, f32(0.0), maxl)
    f1 = np.clip(loc0 + f32(1.0), f32(0.0), maxl)
    w0 = (f1 - loc).astype(np.float32)      # weight for floor corner
    w1 = (f32(1.0) - w0).astype(np.float32)
    i0 = f0.astype(np.int32)
    i1 = f1.astype(np.int32)

    # pass 1 (nearest) map applied to mid index j
    j = np.arange(IN, dtype=np.float32)
    dl = np.clip(j / dz, f32(0.0), f32(IN))
    g = np.clip(np.round(dl), f32(0.0), maxl).astype(np.int32)

    A = np.zeros((IN, RES), dtype=np.float32)
    cols = np.arange(RES)
    A[g[i0], cols] += w0
    A[g[i1], cols] += w1
    return A


def _used_rows(A, lo, n):
    """Indices of rows of A with any nonzero in dst columns [lo, lo+n)."""
    return np.nonzero(np.any(A[:, lo:lo + n] != 0.0, axis=1))[0]


# ----------------------------------------------------------------------------
# Device kernel (built once per process)
# ----------------------------------------------------------------------------

def _build(bench_iters=0):
    key = ("nc", bench_iters)
    if key in _CACHE:
        return _CACHE[key]

    import contextlib

    import concourse.mybir as mybir
    from concourse import bacc, tile

    bf16 = mybir.dt.bfloat16
    f32 = mybir.dt.float32
    nc = bacc.Bacc("TRN2", debug=False)

    slab_d = nc.dram_tensor("slab", (NYC, NXC, NZC), bf16, kind="ExternalInput")
    ay_d = nc.dram_tensor("ay", (NYC, OH), bf16, kind="ExternalInput")
    az_d = nc.dram_tensor("az", (NZC, RES), bf16, kind="ExternalInput")
    # Full-K pass-X weights: axe rows [0, NXC) hold Ax (even j), axo rows
    # [64, 64+NXC) (odd j); all other rows zero.  K=128 matmuls against the
    # full t3 block then pipeline at full-array cadence (no row tiling).
    axe_d = nc.dram_tensor("axe", (128, OH), bf16, kind="ExternalInput")
    axo_d = nc.dram_tensor("axo", (128, OH), bf16, kind="ExternalInput")
    # out columns are j-permuted per 2-quad block: c=8u+4qq+2*eo+h maps to
    # j=4*(2u+qq)+2*h+eo (host unpermutes)
    out_d = nc.dram_tensor("out", (OH, OH, RES), bf16, kind="ExternalOutput")

    CHUNKS = [2, 6, 12, 16]     # slab x-chunk DMA sizes (small first chunk
    assert sum(CHUNKS) == NXC   # so pass Y starts early)
    XG = 4                      # x per pass-Y psum sub-slot
    NU = NQUAD // 2             # 2-quad super-groups (12)

    # build-time greedy balance of psum-evac copies across vector/scalar,
    # using HW-measured per-op costs (PSUM-read bubble included)
    eng_ns = {"v": 0.0, "s": 0.0}

    def evac(nc_, dst, src, n_free):
        if eng_ns["v"] + (n_free + 250) / 0.96 <= eng_ns["s"] + (n_free + 450) / 1.2:
            eng_ns["v"] += (n_free + 250) / 0.96
            nc_.vector.tensor_copy(dst, src)
        else:
            eng_ns["s"] += (n_free + 450) / 1.2
            nc_.scalar.copy(dst, src)

    with tile.TileContext(nc) as tc:
        loop_cm = (
            tc.For_i(0, bench_iters, 1) if bench_iters
            else contextlib.nullcontext()
        )
        with (
            loop_cm,
            tc.tile_pool(name="consts", bufs=1) as consts,
            tc.tile_pool(name="slab", bufs=1) as slab_pool,
            tc.tile_pool(name="t2", bufs=1) as t2_pool,
            tc.tile_pool(name="t3", bufs=3) as t3_pool,
            tc.tile_pool(name="stg", bufs=6) as stg_pool,
        ):
            ay_t = consts.tile([NYC, OH], bf16, tag="ay")
            az_t = consts.tile([NZC, RES], bf16, tag="az")
            axe_t = consts.tile([128, OH], bf16, tag="axe")
            axo_t = consts.tile([128, OH], bf16, tag="axo")

            slabs = []
            x0s = []
            x0 = 0
            for ci, cw in enumerate(CHUNKS):
                s = slab_pool.tile([NYC, cw, NZC], bf16, tag=f"s{ci}")
                slabs.append(s)
                x0s.append(x0)
                x0 += cw
            # chunk 0 and ay gate the first matmul: issue them first on
            # separate queues so they land in parallel
            nc.sync.dma_start(slabs[0][:], slab_d[:, 0:CHUNKS[0], :])
            nc.scalar.dma_start(ay_t[:], ay_d[:])
            nc.sync.dma_start(slabs[1][:], slab_d[:, x0s[1]:x0s[1] + CHUNKS[1], :])
            nc.scalar.dma_start(slabs[2][:], slab_d[:, x0s[2]:x0s[2] + CHUNKS[2], :])
            nc.sync.dma_start(slabs[3][:], slab_d[:, x0s[3]:x0s[3] + CHUNKS[3], :])
            nc.scalar.dma_start(az_t[:], az_d[:])
            nc.sync.dma_start(axe_t[:], axe_d[:])
            nc.scalar.dma_start(axo_t[:], axo_d[:])

            # t2[z, j, x] with 64-wide x: pad cols [NXC, XP) zeroed once so
            # pass-Z weight blocks (and hence t3 pad rows) are exact zeros
            t2 = t2_pool.tile([NZC, OH, XP], bf16, tag="t2")
            nc.gpsimd.memset(t2[:, :, NXC:XP], 0.0)

            def chunk_of(x):
                for ci, lo in enumerate(x0s):
                    if lo <= x < lo + CHUNKS[ci]:
                        return ci, x - lo
                raise AssertionError(x)

            # ---- pass Y: t2[z, j, x] = sum_y slab[y, x, z] * Ay[y, j] ----
            # psum supers span 2 banks (8 x-slices; bank-aligned 512B slots)
            with tc.tile_pool(name="psY", bufs=3, space="PSUM") as psy_pool:
                for xg in range(NXC // (2 * XG)):
                    ps = psy_pool.tile([NZC, 2 * XG, 128], f32, tag="psY8")
                    for xi in range(2 * XG):
                        x = xg * 2 * XG + xi
                        ci, xl = chunk_of(x)
                        nc.tensor.matmul(
                            ps[:, xi, 0:OH], slabs[ci][:, xl, :], ay_t[:]
                        )
                    dst = t2[:, :, xg * 2 * XG:(xg + 1) * 2 * XG]
                    evac(nc, dst, ps[:, :, 0:OH].transpose([0, 2, 1]), 2 * XG * OH)
                # ragged tail (4 slices)
                ps = psy_pool.tile([NZC, XG, 128], f32, tag="psY4", bufs=1)
                for xi in range(XG):
                    x = (NXC // (2 * XG)) * 2 * XG + xi
                    ci, xl = chunk_of(x)
                    nc.tensor.matmul(ps[:, xi, 0:OH], slabs[ci][:, xl, :], ay_t[:])
                dst = t2[:, :, NXC - XG:NXC]
                evac(nc, dst, ps[:, :, 0:OH].transpose([0, 2, 1]), XG * OH)

            # ---- pass Z + pass X, per 2-quad super-group (8 j) ----
            # software-pipelined one group deep: Z-MMs of group u issue
            # before X-MMs of group u-1, so an X stall (psum-evac wait)
            # never blocks the independent Z stream behind it
            with (
                tc.tile_pool(name="psZ", bufs=2, space="PSUM") as psz_pool,
                tc.tile_pool(name="psX", bufs=2, space="PSUM") as psx_pool,
            ):
                dma_flip = [0]

                def emit_x(t3, u):
                    stg = stg_pool.tile([OH, 8, RES], bf16, tag="stg")
                    for qq in range(2):
                        # pass X: full-K matmuls over the quad's t3 block
                        px = psx_pool.tile([OH, 2, 512], f32, tag="px")
                        rhs = t3[:, 2 * qq:2 * qq + 2, :]
                        nc.tensor.matmul(px[:, 0, 0:2 * RES], axe_t[:], rhs)
                        nc.tensor.matmul(px[:, 1, 0:2 * RES], axo_t[:], rhs)
                        c0 = 8 * u + 4 * qq
                        if u == NU - 1:
                            # drain the tail fast: parallel half-evacs and
                            # 2-column stores on both store queues
                            nc.vector.tensor_copy(
                                stg[:, 4 * qq:4 * qq + 2, :], px[:, 0, 0:2 * RES]
                            )
                            nc.scalar.copy(
                                stg[:, 4 * qq + 2:4 * qq + 4, :], px[:, 1, 0:2 * RES]
                            )
                            nc.sync.dma_start(
                                out_d[:, c0:c0 + 2, :], stg[:, 4 * qq:4 * qq + 2, :]
                            )
                            nc.gpsimd.dma_start(
                                out_d[:, c0 + 2:c0 + 4, :],
                                stg[:, 4 * qq + 2:4 * qq + 4, :],
                            )
                            continue
                        dst = stg[:, 4 * qq:4 * qq + 4, :]
                        evac(nc, dst, px[:, :, 0:2 * RES], 4 * RES)
                        # store per quad (permuted j; host unshuffles)
                        eng = nc.sync if dma_flip[0] % 2 == 0 else nc.gpsimd
                        dma_flip[0] += 1
                        eng.dma_start(
                            out_d[:, c0:c0 + 4, :],
                            stg[:, 4 * qq:4 * qq + 4, :],
                        )

                prev = None
                for u in range(NU):
                    # pass Z: 4 pairs -> psZ super [128, 4, 192] (2 banks)
                    pz = psz_pool.tile([128, 4, 256], f32, tag="pz")
                    for s in range(4):
                        p = 4 * u + s
                        nc.tensor.matmul(
                            pz[:, s, 0:RES], t2[:, 2 * p:2 * p + 2, :], az_t[:]
                        )
                    t3 = t3_pool.tile([128, 4, RES], bf16, tag="t3")
                    evac(nc, t3[:], pz[:, :, 0:RES], 4 * RES)
                    if prev is not None:
                        emit_x(*prev)
                    prev = (t3, u)
                emit_x(*prev)

    nc.compile()
    _CACHE[key] = nc
    return nc


# ----------------------------------------------------------------------------
# Host wrapper
# ----------------------------------------------------------------------------

def _in_maps(vol, sub):
    """Compact per-core inputs; returns None if any compacted dim overflows."""
    import ml_dtypes

    bf16 = ml_dtypes.bfloat16
    maps = []
    spans = []
    tabs = {}
    for core in range(8):
        b = core >> 2
        ix = (core >> 1) & 1
        iy = core & 1
        if b not in tabs:
            tabs[b] = tuple(_axis_matrix(sub[b, d]) for d in range(3))
        Ax, Ay, Az = tabs[b]
        xr = _used_rows(Ax, ix * OH, OH)
        yr = _used_rows(Ay, iy * OH, OH)
        zr = _used_rows(Az, 0, RES)
        if len(xr) > NXC or len(yr) > NYC or len(zr) > NZC:
            return None, None
        axc = Ax[xr][:, ix * OH:(ix + 1) * OH]
        axe = np.zeros((128, OH), dtype=np.float32)
        axe[0:len(xr)] = axc
        axo = np.zeros((128, OH), dtype=np.float32)
        axo[64:64 + len(xr)] = axc
        ayp = np.zeros((NYC, OH), dtype=np.float32)
        ayp[0:len(yr)] = Ay[yr][:, iy * OH:(iy + 1) * OH]
        azp = np.zeros((NZC, RES), dtype=np.float32)
        azp[0:len(zr)] = Az[zr]
        slab = np.zeros((NYC, NXC, NZC), dtype=np.float32)
        slab[0:len(yr), 0:len(xr), 0:len(zr)] = (
            vol[b, :, :, :, 0][np.ix_(xr, yr, zr)].transpose(1, 0, 2)
        )
        maps.append({
            "slab": slab.astype(bf16),
            "axe": axe.astype(bf16),
            "axo": axo.astype(bf16),
            "ay": ayp.astype(bf16),
            "az": azp.astype(bf16),
        })
        spans.append((b, ix * OH, iy * OH))
    return maps, spans


def kernel(vol, subsample_res):
    global LAST_RESULTS
    from concourse import bass_utils

    vol = np.asarray(vol, dtype=np.float32)
    sub = np.asarray(subsample_res, dtype=np.float32)
    maps, spans = _in_maps(vol, sub)
    if maps is None:
        return _kernel_fallback(vol, sub)
    nc = _build()
    res = bass_utils.run_bass_kernel_spmd(nc, maps, core_ids=list(range(8)))
    LAST_RESULTS = res
    # out column c = 8u + 4qq + 2*eo + h holds j = 4*(2u+qq) + 2*h + eo
    jmap = np.empty(OH, dtype=np.int64)
    for q in range(NQUAD):
        for eo in range(2):
            for h in range(2):
                jmap[4 * q + 2 * eo + h] = 4 * q + 2 * h + eo
    out = np.empty((2, RES, RES, RES, 1), dtype=np.float32)
    for core, (b, x0, y0) in enumerate(spans):
        blk = np.asarray(res.results[core]["out"], dtype=np.float32)
        out[b, x0:x0 + OH, y0 + jmap, :, 0] = blk.transpose(1, 0, 2)
    return out


# ----------------------------------------------------------------------------
# Fallback: original banded-slab kernel (handles any subsample_res in [1, 4])
# ----------------------------------------------------------------------------

H = 112           # padded source-band rows for the sharded axes (x, y)
XPF = H
Z = 192
C0 = 122
Z1LO = 116
Z1N = Z - Z1LO


def _band_slice(A, lo, n):
    cols = A[:, lo:lo + n]
    rows = np.nonzero(np.any(cols != 0.0, axis=1))[0]
    rmin, rmax = int(rows[0]), int(rows[-1])
    assert rmax - rmin + 1 <= H, f"band too wide: {rmax - rmin + 1}"
    S0 = min(rmin, IN - H)
    assert rmax < S0 + H
    return S0, np.ascontiguousarray(cols[S0:S0 + H])


def _build_fb(bench_iters=0):
    key = ("nc_fb", bench_iters)
    if key in _CACHE:
        return _CACHE[key]

    import contextlib

    import concourse.mybir as mybir
    from concourse import bacc, tile

    bf16 = mybir.dt.bfloat16
    nc = bacc.Bacc("TRN2", debug=False)

    slab_d = nc.dram_tensor("slab", (H, H, Z), bf16, kind="ExternalInput")
    ax_d = nc.dram_tensor("ax", (XPF, OH), bf16, kind="ExternalInput")
    ay_d = nc.dram_tensor("ay", (H, OH), bf16, kind="ExternalInput")
    az0_d = nc.dram_tensor("az0", (128, C0), bf16, kind="ExternalInput")
    az1_d = nc.dram_tensor("az1", (Z1N, RES - C0), bf16, kind="ExternalInput")
    out_d = nc.dram_tensor("out", (OH, OH, Z), bf16, kind="ExternalOutput")

    CHUNKS = [4, 8, 12, 24, 32, 32]
    assert sum(CHUNKS) == H
    XG = 4
    JB = 8
    NB = OH // JB

    with tile.TileContext(nc) as tc:
        loop_cm = (
            tc.For_i(0, bench_iters, 1) if bench_iters
            else contextlib.nullcontext()
        )
        with (
            loop_cm,
            tc.tile_pool(name="consts", bufs=1) as consts,
            tc.tile_pool(name="slab", bufs=1) as slab_pool,
            tc.tile_pool(name="t2", bufs=1) as t2_pool,
            tc.tile_pool(name="t3", bufs=2) as t3_pool,
            tc.tile_pool(name="stage", bufs=3) as stage_pool,
        ):
            ay_t = consts.tile([H, OH], bf16, tag="ay")
            az0_t = consts.tile([128, C0], bf16, tag="az0")
            az1_t = consts.tile([Z1N, RES - C0], bf16, tag="az1")
            ax_t = consts.tile([XPF, OH], bf16, tag="ax")

            slabs = []
            x0s = []
            x0 = 0
            for ci, cw in enumerate(CHUNKS):
                s = slab_pool.tile([H, cw, Z], bf16, tag=f"s{ci}")
                slabs.append(s)
                x0s.append(x0)
                x0 += cw
            nc.sync.dma_start(slabs[0][:], slab_d[:, 0:CHUNKS[0], :])
            nc.scalar.dma_start(slabs[1][:], slab_d[:, x0s[1]:x0s[1] + CHUNKS[1], :])
            nc.sync.dma_start(ay_t[:], ay_d[:])
            nc.scalar.dma_start(az0_t[:], az0_d[:])
            nc.scalar.dma_start(az1_t[:], az1_d[:])
            nc.scalar.dma_start(ax_t[:], ax_d[:])
            for ci in range(2, len(CHUNKS)):
                eng = nc.sync if ci % 2 == 0 else nc.scalar
                eng.dma_start(
                    slabs[ci][:], slab_d[:, x0s[ci]:x0s[ci] + CHUNKS[ci], :]
                )

            t2a = t2_pool.tile([128, OH, XPF], bf16, tag="t2a")
            t2b = t2_pool.tile([Z1N, OH, XPF], bf16, tag="t2b")

            def chunk_of(x):
                for ci, lo in enumerate(x0s):
                    if lo <= x < lo + CHUNKS[ci]:
                        return ci, x - lo
                raise AssertionError(x)

            with (
                tc.tile_pool(name="psumw", bufs=1, space="PSUM") as psumw,
                tc.tile_pool(name="psum1", bufs=3, space="PSUM") as psum1,
            ):
                warm = consts.tile([1, 512], bf16, tag="warm")
                nc.gpsimd.memset(warm[:], 0.0)
                psw = psumw.tile([128, 512], mybir.dt.float32, tag="psw")
                for _ in range(30):
                    nc.tensor.matmul(psw[:], warm[:, 0:128], warm[:])

                for xg in range(H // XG):
                    psA = psum1.tile([128, XG, OH], mybir.dt.float32, tag="psA")
                    psB = psum1.tile([Z1N, XG, OH], mybir.dt.float32, tag="psB")
                    for xi in range(XG):
                        x = xg * XG + xi
                        ci, xl = chunk_of(x)
                        s = slabs[ci]
                        nc.tensor.matmul(psA[:, xi, :], s[:, xl, 0:128], ay_t[:])
                        nc.tensor.matmul(psB[:, xi, :], s[:, xl, Z1LO:Z], ay_t[:])
                    lo = xg * XG
                    dstA = t2a[:, :, lo:lo + XG]
                    dstB = t2b[:, :, lo:lo + XG]
                    srcA = psA[:].transpose([0, 2, 1])
                    srcB = psB[:].transpose([0, 2, 1])
                    if xg % 2 == 0:
                        nc.scalar.copy(dstA, srcA)
                        nc.vector.tensor_copy(dstB, srcB)
                    else:
                        nc.vector.tensor_copy(dstA, srcA)
                        nc.scalar.copy(dstB, srcB)

            with tc.tile_pool(name="psum2", bufs=3, space="PSUM") as psum2:
                for jb in range(NB):
                    t3 = t3_pool.tile([XPF, JB, Z], bf16, tag="t3")
                    stage = stage_pool.tile([OH, JB, Z], bf16, tag="st")
                    for jg in range(JB // 2):
                        pz = psum2.tile([XPF, 2, RES], mybir.dt.float32, tag="pz")
                        for ji in range(2):
                            j = jb * JB + jg * 2 + ji
                            nc.tensor.matmul(pz[:, ji, 0:C0], t2a[:, j, :], az0_t[:])
                            nc.tensor.matmul(pz[:, ji, C0:RES], t2b[:, j, :], az1_t[:])
                        t3d = t3[:, jg * 2:jg * 2 + 2, :]
                        px = psum2.tile([OH, 2, Z], mybir.dt.float32, tag="px")
                        std = stage[:, jg * 2:jg * 2 + 2, :]
                        if jg % 2 == 0:
                            nc.scalar.copy(t3d, pz[:])
                            nc.tensor.matmul(px[:], ax_t[:], t3d)
                            nc.vector.tensor_copy(std, px[:])
                        else:
                            nc.vector.tensor_copy(t3d, pz[:])
                            nc.tensor.matmul(px[:], ax_t[:], t3d)
                            nc.scalar.copy(std, px[:])
                    j0 = jb * JB
                    if jb == NB - 1:
                        qj = JB // 4
                        for q in range(4):
                            eng = nc.gpsimd if q % 2 == 0 else nc.sync
                            eng.dma_start(
                                out_d[:, j0 + q * qj:j0 + (q + 1) * qj, :],
                                stage[:, q * qj:(q + 1) * qj, :],
                            )
                    elif jb % 2 == 0:
                        nc.gpsimd.dma_start(out_d[:, j0:j0 + JB, :], stage[:])
                    else:
                        nc.sync.dma_start(out_d[:, j0:j0 + JB, :], stage[:])

    nc.compile()
    _CACHE[key] = nc
    return nc


def _in_maps_fb(vol, sub):
    import ml_dtypes

    bf16 = ml_dtypes.bfloat16
    maps = []
    spans = []
    tabs = {}
    for core in range(8):
        b = core >> 2
        ix = (core >> 1) & 1
        iy = core & 1
        if b not in tabs:
            tabs[b] = tuple(_axis_matrix(sub[b, d]) for d in range(3))
        Ax, Ay, Az = tabs[b]
        SX, axs = _band_slice(Ax, ix * OH, OH)
        SY, ays = _band_slice(Ay, iy * OH, OH)
        axp = np.zeros((XPF, OH), dtype=np.float32)
        axp[:H] = axs
        assert not Az[128:, :C0].any(), "az0 band bound violated"
        assert not Az[:Z1LO, C0:].any(), "az1 band bound violated"
        az0 = np.ascontiguousarray(Az[0:128, 0:C0])
        az1 = np.ascontiguousarray(Az[Z1LO:Z, C0:RES])
        slab = np.ascontiguousarray(
            vol[b, SX:SX + H, SY:SY + H, :, 0].transpose(1, 0, 2)
        )
        maps.append({
            "slab": slab.astype(bf16),
            "ax": axp.astype(bf16),
            "ay": ays.astype(bf16),
            "az0": az0.astype(bf16),
            "az1": az1.astype(bf16),
        })
        spans.append((b, ix * OH, iy * OH))
    return maps, spans


def _kernel_fallback(vol, sub):
    global LAST_RESULTS
    from concourse import bass_utils

    nc = _build_fb()
    maps, spans = _in_maps_fb(vol, sub)
    res = bass_utils.run_bass_kernel_spmd(nc, maps, core_ids=list(range(8)))
    LAST_RESULTS = res
    out = np.empty((2, RES, RES, RES, 1), dtype=np.float32)
    for core, (b, x0, y0) in enumerate(spans):
        out[b, x0:x0 + OH, y0:y0 + OH, :, 0] = np.asarray(
            res.results[core]["out"], dtype=np.float32
        )
    return out


# revision 16
# speedup vs baseline: 1.1664x; 1.1095x over previous
"""MimicAcquisition (double resample: nearest-at-acquisition-res then trilinear
back) as three separable banded-matrix contractions on the PE engine, in bf16,
with *used-row compaction*.

out[i,j,k] = sum_{a,b,c} Ax[a,i] * Ay[b,j] * Az[c,k] * vol[a,b,c]

A_d = (trilinear upsample) @ (nearest resample) along axis d.  The nearest
resample for subsample_res r reads only ~192/r distinct source rows, so A_d
has few nonzero rows.  Host gathers exactly those rows: the per-core slab
shrinks from 112x112x192 to <=80 x <=36 x <=88 (seed-0 inputs), cutting input
DMA ~9.5x and all matmul contractions proportionally.

Each of the 8 cores handles one (batch, i-half, j-half) octant.  Device:

  pass Y: t2[z, j, x]   = sum_y slab[y, x, z] * Ay[y, j]      (per-x weights)
  pass Z: t3[(j2,x), k] = sum_z t2[z, j-pair, x] * Az[z, k]   (128-col FWL)
  pass X: out[i, j, k]  = sum_x Ax[x, i] * t3[x-slice, k]     (row+col tiled)

t2 packs j-pairs at a 64-element x stride so pass-Z weights are a contiguous
128-column block and the psZ partition layout (j0 @ 0-63, j1 @ 64-127) feeds
pass X with 32-aligned rhs partition bases.  Pass X splits i into [0,64) and
[64,96) blocks at four array tile positions, so psum tiles carry 128 useful
partitions and the two row strips run concurrently.  PSUM evacuation (the
bottleneck: DVE/ACT are the only PSUM readers, 1 elem/cycle/partition) is
greedily balanced across vector+scalar by a build-time cost model.
"""

import sys

if "/opt/trn_rl_repo" not in sys.path:
    sys.path.insert(0, "/opt/trn_rl_repo")

import numpy as np

IN = 192          # input extent per axis
RES = 192         # resample (output) extent per axis
OH = 96           # output half extent for sharded axes (i, j)
NYC = 80          # compacted y rows (max used: 77)
NXC = 36          # compacted x rows per i-half (max used: 35)
NZC = 88          # compacted z rows, full axis (max used: 85)
XP = 64           # x stride in t2: 2 j per 128-column weight block
NPAIR = OH // 2   # 48 j-pairs
NQUAD = OH // 4   # 24 j-quads

_CACHE = {}

LAST_RESULTS = None


# ----------------------------------------------------------------------------
# Host-side table construction (mirrors reference.py float32 arithmetic)
# ----------------------------------------------------------------------------

def _axis_matrix(r):
    """A[src, dst] for one axis given subsample resolution r (float32)."""
    f32 = np.float32
    d = (f32(IN) * f32(1.0) / f32(r)).astype(np.int32)  # down_shape (trunc)
    dz = f32(d) / f32(IN)                               # down_zoom
    uz = f32(RES) / f32(d)                              # up_zoom
    maxl = f32(IN - 1)

    # pass 2 (trilinear) locations for output index i, in mid coordinates
    i = np.arange(RES, dtype=np.float32)
    loc = np.clip(i / uz, f32(0.0), maxl)
    loc0 = np.floor(loc)
    f0 = np.clip(loc0, f32(0.0), maxl)
    f1 = np.clip(loc0 + f32(1.0), f32(0.0), maxl)
    w0 = (f1 - loc).astype(np.float32)      # weight for floor corner
    w1 = (f32(1.0) - w0).astype(np.float32)
    i0 = f0.astype(np.int32)
    i1 = f1.astype(np.int32)

    # pass 1 (nearest) map applied to mid index j
    j = np.arange(IN, dtype=np.float32)
    dl = np.clip(j / dz, f32(0.0), f32(IN))
    g = np.clip(np.round(dl), f32(0.0), maxl).astype(np.int32)

    A = np.zeros((IN, RES), dtype=np.float32)
    cols = np.arange(RES)
    A[g[i0], cols] += w0
    A[g[i1], cols] += w1
    return A


def _used_rows(A, lo, n):
    """Indices of rows of A with any nonzero in dst columns [lo, lo+n)."""
    return np.nonzero(np.any(A[:, lo:lo + n] != 0.0, axis=1))[0]


# ----------------------------------------------------------------------------
# Device kernel (built once per process)
# ----------------------------------------------------------------------------

def _build(bench_iters=0):
    key = ("nc", bench_iters)
    if key in _CACHE:
        return _CACHE[key]

    import contextlib

    import concourse.mybir as mybir
    from concourse import bacc, tile

    bf16 = mybir.dt.bfloat16
    f32 = mybir.dt.float32
    nc = bacc.Bacc("TRN2", debug=False)

    slab_d = nc.dram_tensor("slab", (NYC, NXC, NZC), bf16, kind="ExternalInput")
    ay_d = nc.dram_tensor("ay", (NYC, OH), bf16, kind="ExternalInput")
    az_d = nc.dram_tensor("az", (NZC, RES), bf16, kind="ExternalInput")
    # Full-K pass-X weights: axe rows [0, NXC) hold Ax (even j), axo rows
    # [64, 64+NXC) (odd j); all other rows zero.  K=128 matmuls against the
    # full t3 block then pipeline at full-array cadence (no row tiling).
    axe_d = nc.dram_tensor("axe", (128, OH), bf16, kind="ExternalInput")
    axo_d = nc.dram_tensor("axo", (128, OH), bf16, kind="ExternalInput")
    # out columns are j-permuted per 2-quad block: c=8u+4qq+2*eo+h maps to
    # j=4*(2u+qq)+2*h+eo (host unpermutes)
    out_d = nc.dram_tensor("out", (OH, OH, RES), bf16, kind="ExternalOutput")

    CHUNKS = [2, 6, 12, 16]     # slab x-chunk DMA sizes (small first chunk
    assert sum(CHUNKS) == NXC   # so pass Y starts early)
    XG = 4                      # x per pass-Y psum sub-slot
    NU = NQUAD // 2             # 2-quad super-groups (12)

    # build-time greedy balance of psum-evac copies across vector/scalar,
    # using HW-measured per-op costs (PSUM-read bubble included)
    eng_ns = {"v": 0.0, "s": 0.0}

    def evac(nc_, dst, src, n_free):
        if eng_ns["v"] + (n_free + 250) / 0.96 <= eng_ns["s"] + (n_free + 450) / 1.2:
            eng_ns["v"] += (n_free + 250) / 0.96
            nc_.vector.tensor_copy(dst, src)
        else:
            eng_ns["s"] += (n_free + 450) / 1.2
            nc_.scalar.copy(dst, src)

    with tile.TileContext(nc) as tc:
        loop_cm = (
            tc.For_i(0, bench_iters, 1) if bench_iters
            else contextlib.nullcontext()
        )
        with (
            loop_cm,
            tc.tile_pool(name="consts", bufs=1) as consts,
            tc.tile_pool(name="slab", bufs=1) as slab_pool,
            tc.tile_pool(name="t2", bufs=1) as t2_pool,
            tc.tile_pool(name="t3", bufs=4) as t3_pool,
            tc.tile_pool(name="stg", bufs=8) as stg_pool,
        ):
            ay_t = consts.tile([NYC, OH], bf16, tag="ay")
            az_t = consts.tile([NZC, RES], bf16, tag="az")
            axe_t = consts.tile([128, OH], bf16, tag="axe")
            axo_t = consts.tile([128, OH], bf16, tag="axo")

            slabs = []
            x0s = []
            x0 = 0
            for ci, cw in enumerate(CHUNKS):
                s = slab_pool.tile([NYC, cw, NZC], bf16, tag=f"s{ci}")
                slabs.append(s)
                x0s.append(x0)
                x0 += cw
            # chunk 0 and ay gate the first matmul: issue them first on
            # separate queues so they land in parallel
            nc.sync.dma_start(slabs[0][:], slab_d[:, 0:CHUNKS[0], :])
            nc.scalar.dma_start(ay_t[:], ay_d[:])
            nc.sync.dma_start(slabs[1][:], slab_d[:, x0s[1]:x0s[1] + CHUNKS[1], :])
            nc.scalar.dma_start(slabs[2][:], slab_d[:, x0s[2]:x0s[2] + CHUNKS[2], :])
            nc.sync.dma_start(slabs[3][:], slab_d[:, x0s[3]:x0s[3] + CHUNKS[3], :])
            nc.scalar.dma_start(az_t[:], az_d[:])
            nc.sync.dma_start(axe_t[:], axe_d[:])
            nc.scalar.dma_start(axo_t[:], axo_d[:])

            # t2[z, j, x] with 64-wide x: pad cols [NXC, XP) zeroed once so
            # pass-Z weight blocks (and hence t3 pad rows) are exact zeros
            t2 = t2_pool.tile([NZC, OH, XP], bf16, tag="t2")
            nc.gpsimd.memset(t2[:, :, NXC:XP], 0.0)

            def chunk_of(x):
                for ci, lo in enumerate(x0s):
                    if lo <= x < lo + CHUNKS[ci]:
                        return ci, x - lo
                raise AssertionError(x)

            # ---- pass Y: t2[z, j, x] = sum_y slab[y, x, z] * Ay[y, j] ----
            # psum supers span 2 banks (8 x-slices; bank-aligned 512B slots)
            with tc.tile_pool(name="psY", bufs=3, space="PSUM") as psy_pool:
                for xg in range(NXC // (2 * XG)):
                    ps = psy_pool.tile([NZC, 2 * XG, 128], f32, tag="psY8")
                    for xi in range(2 * XG):
                        x = xg * 2 * XG + xi
                        ci, xl = chunk_of(x)
                        nc.tensor.matmul(
                            ps[:, xi, 0:OH], slabs[ci][:, xl, :], ay_t[:]
                        )
                    dst = t2[:, :, xg * 2 * XG:(xg + 1) * 2 * XG]
                    evac(nc, dst, ps[:, :, 0:OH].transpose([0, 2, 1]), 2 * XG * OH)
                # ragged tail (4 slices)
                ps = psy_pool.tile([NZC, XG, 128], f32, tag="psY4", bufs=1)
                for xi in range(XG):
                    x = (NXC // (2 * XG)) * 2 * XG + xi
                    ci, xl = chunk_of(x)
                    nc.tensor.matmul(ps[:, xi, 0:OH], slabs[ci][:, xl, :], ay_t[:])
                dst = t2[:, :, NXC - XG:NXC]
                evac(nc, dst, ps[:, :, 0:OH].transpose([0, 2, 1]), XG * OH)

            # ---- pass Z + pass X, per quad (4 j = 2 pairs) ----
            # software-pipelined one quad deep: Z-MMs of quad q issue before
            # X-MMs of quad q-1, so an X stall (psum-evac wait) never blocks
            # the independent Z stream behind it
            with (
                tc.tile_pool(name="psZ", bufs=2, space="PSUM") as psz_pool,
                tc.tile_pool(name="psX", bufs=3, space="PSUM") as psx_pool,
            ):
                dma_flip = [0]

                def emit_x(t3, q):
                    stg = stg_pool.tile([OH, 4, RES], bf16, tag="stg")
                    # pass X: full-K matmuls over the quad's t3 block
                    px = psx_pool.tile([OH, 2, 512], f32, tag="px")
                    nc.tensor.matmul(px[:, 0, 0:2 * RES], axe_t[:], t3[:])
                    nc.tensor.matmul(px[:, 1, 0:2 * RES], axo_t[:], t3[:])
                    c0 = 4 * q
                    if q == NQUAD - 1:
                        # drain the tail fast: parallel half-evacs and
                        # 2-column stores on both store queues
                        nc.vector.tensor_copy(
                            stg[:, 0:2, :], px[:, 0, 0:2 * RES]
                        )
                        nc.scalar.copy(
                            stg[:, 2:4, :], px[:, 1, 0:2 * RES]
                        )
                        nc.sync.dma_start(out_d[:, c0:c0 + 2, :], stg[:, 0:2, :])
                        nc.gpsimd.dma_start(
                            out_d[:, c0 + 2:c0 + 4, :], stg[:, 2:4, :]
                        )
                        return
                    evac(nc, stg[:], px[:, :, 0:2 * RES], 4 * RES)
                    # store per quad (permuted j; host unshuffles)
                    eng = nc.sync if dma_flip[0] % 2 == 0 else nc.gpsimd
                    dma_flip[0] += 1
                    eng.dma_start(out_d[:, c0:c0 + 4, :], stg[:])

                prev = None
                for q in range(NQUAD):
                    # pass Z: 2 pairs -> psZ tile [128, 2, 192] (1 bank)
                    pz = psz_pool.tile([128, 2, 256], f32, tag="pz")
                    for s in range(2):
                        p = 2 * q + s
                        nc.tensor.matmul(
                            pz[:, s, 0:RES], t2[:, 2 * p:2 * p + 2, :], az_t[:]
                        )
                    t3 = t3_pool.tile([128, 2, RES], bf16, tag="t3")
                    evac(nc, t3[:], pz[:, :, 0:RES], 2 * RES)
                    if prev is not None:
                        emit_x(*prev)
                    prev = (t3, q)
                emit_x(*prev)

    nc.compile()
    _CACHE[key] = nc
    return nc


# ----------------------------------------------------------------------------
# Host wrapper
# ----------------------------------------------------------------------------

def _in_maps(vol, sub):
    """Compact per-core inputs; returns None if any compacted dim overflows."""
    import ml_dtypes

    bf16 = ml_dtypes.bfloat16
    maps = []
    spans = []
    tabs = {}
    for core in range(8):
        b = core >> 2
        ix = (core >> 1) & 1
        iy = core & 1
        if b not in tabs:
            tabs[b] = tuple(_axis_matrix(sub[b, d]) for d in range(3))
        Ax, Ay, Az = tabs[b]
        xr = _used_rows(Ax, ix * OH, OH)
        yr = _used_rows(Ay, iy * OH, OH)
        zr = _used_rows(Az, 0, RES)
        if len(xr) > NXC or len(yr) > NYC or len(zr) > NZC:
            return None, None
        axc = Ax[xr][:, ix * OH:(ix + 1) * OH]
        axe = np.zeros((128, OH), dtype=np.float32)
        axe[0:len(xr)] = axc
        axo = np.zeros((128, OH), dtype=np.float32)
        axo[64:64 + len(xr)] = axc
        ayp = np.zeros((NYC, OH), dtype=np.float32)
        ayp[0:len(yr)] = Ay[yr][:, iy * OH:(iy + 1) * OH]
        azp = np.zeros((NZC, RES), dtype=np.float32)
        azp[0:len(zr)] = Az[zr]
        slab = np.zeros((NYC, NXC, NZC), dtype=np.float32)
        slab[0:len(yr), 0:len(xr), 0:len(zr)] = (
            vol[b, :, :, :, 0][np.ix_(xr, yr, zr)].transpose(1, 0, 2)
        )
        maps.append({
            "slab": slab.astype(bf16),
            "axe": axe.astype(bf16),
            "axo": axo.astype(bf16),
            "ay": ayp.astype(bf16),
            "az": azp.astype(bf16),
        })
        spans.append((b, ix * OH, iy * OH))
    return maps, spans


def kernel(vol, subsample_res):
    global LAST_RESULTS
    from concourse import bass_utils

    vol = np.asarray(vol, dtype=np.float32)
    sub = np.asarray(subsample_res, dtype=np.float32)
    maps, spans = _in_maps(vol, sub)
    if maps is None:
        return _kernel_fallback(vol, sub)
    nc = _build()
    res = bass_utils.run_bass_kernel_spmd(nc, maps, core_ids=list(range(8)))
    LAST_RESULTS = res
    # out column c = 8u + 4qq + 2*eo + h holds j = 4*(2u+qq) + 2*h + eo
    jmap = np.empty(OH, dtype=np.int64)
    for q in range(NQUAD):
        for eo in range(2):
            for h in range(2):
                jmap[4 * q + 2 * eo + h] = 4 * q + 2 * h + eo
    out = np.empty((2, RES, RES, RES, 1), dtype=np.float32)
    for core, (b, x0, y0) in enumerate(spans):
        blk = np.asarray(res.results[core]["out"], dtype=np.float32)
        out[b, x0:x0 + OH, y0 + jmap, :, 0] = blk.transpose(1, 0, 2)
    return out


# ----------------------------------------------------------------------------
# Fallback: original banded-slab kernel (handles any subsample_res in [1, 4])
# ----------------------------------------------------------------------------

H = 112           # padded source-band rows for the sharded axes (x, y)
XPF = H
Z = 192
C0 = 122
Z1LO = 116
Z1N = Z - Z1LO


def _band_slice(A, lo, n):
    cols = A[:, lo:lo + n]
    rows = np.nonzero(np.any(cols != 0.0, axis=1))[0]
    rmin, rmax = int(rows[0]), int(rows[-1])
    assert rmax - rmin + 1 <= H, f"band too wide: {rmax - rmin + 1}"
    S0 = min(rmin, IN - H)
    assert rmax < S0 + H
    return S0, np.ascontiguousarray(cols[S0:S0 + H])


def _build_fb(bench_iters=0):
    key = ("nc_fb", bench_iters)
    if key in _CACHE:
        return _CACHE[key]

    import contextlib

    import concourse.mybir as mybir
    from concourse import bacc, tile

    bf16 = mybir.dt.bfloat16
    nc = bacc.Bacc("TRN2", debug=False)

    slab_d = nc.dram_tensor("slab", (H, H, Z), bf16, kind="ExternalInput")
    ax_d = nc.dram_tensor("ax", (XPF, OH), bf16, kind="ExternalInput")
    ay_d = nc.dram_tensor("ay", (H, OH), bf16, kind="ExternalInput")
    az0_d = nc.dram_tensor("az0", (128, C0), bf16, kind="ExternalInput")
    az1_d = nc.dram_tensor("az1", (Z1N, RES - C0), bf16, kind="ExternalInput")
    out_d = nc.dram_tensor("out", (OH, OH, Z), bf16, kind="ExternalOutput")

    CHUNKS = [4, 8, 12, 24, 32, 32]
    assert sum(CHUNKS) == H
    XG = 4
    JB = 8
    NB = OH // JB

    with tile.TileContext(nc) as tc:
        loop_cm = (
            tc.For_i(0, bench_iters, 1) if bench_iters
            else contextlib.nullcontext()
        )
        with (
            loop_cm,
            tc.tile_pool(name="consts", bufs=1) as consts,
            tc.tile_pool(name="slab", bufs=1) as slab_pool,
            tc.tile_pool(name="t2", bufs=1) as t2_pool,
            tc.tile_pool(name="t3", bufs=2) as t3_pool,
            tc.tile_pool(name="stage", bufs=3) as stage_pool,
        ):
            ay_t = consts.tile([H, OH], bf16, tag="ay")
            az0_t = consts.tile([128, C0], bf16, tag="az0")
            az1_t = consts.tile([Z1N, RES - C0], bf16, tag="az1")
            ax_t = consts.tile([XPF, OH], bf16, tag="ax")

            slabs = []
            x0s = []
            x0 = 0
            for ci, cw in enumerate(CHUNKS):
                s = slab_pool.tile([H, cw, Z], bf16, tag=f"s{ci}")
                slabs.append(s)
                x0s.append(x0)
                x0 += cw
            nc.sync.dma_start(slabs[0][:], slab_d[:, 0:CHUNKS[0], :])
            nc.scalar.dma_start(slabs[1][:], slab_d[:, x0s[1]:x0s[1] + CHUNKS[1], :])
            nc.sync.dma_start(ay_t[:], ay_d[:])
            nc.scalar.dma_start(az0_t[:], az0_d[:])
            nc.scalar.dma_start(az1_t[:], az1_d[:])
            nc.scalar.dma_start(ax_t[:], ax_d[:])
            for ci in range(2, len(CHUNKS)):
                eng = nc.sync if ci % 2 == 0 else nc.scalar
                eng.dma_start(
                    slabs[ci][:], slab_d[:, x0s[ci]:x0s[ci] + CHUNKS[ci], :]
                )

            t2a = t2_pool.tile([128, OH, XPF], bf16, tag="t2a")
            t2b = t2_pool.tile([Z1N, OH, XPF], bf16, tag="t2b")

            def chunk_of(x):
                for ci, lo in enumerate(x0s):
                    if lo <= x < lo + CHUNKS[ci]:
                        return ci, x - lo
                raise AssertionError(x)

            with (
                tc.tile_pool(name="psumw", bufs=1, space="PSUM") as psumw,
                tc.tile_pool(name="psum1", bufs=3, space="PSUM") as psum1,
            ):
                warm = consts.tile([1, 512], bf16, tag="warm")
                nc.gpsimd.memset(warm[:], 0.0)
                psw = psumw.tile([128, 512], mybir.dt.float32, tag="psw")
                for _ in range(30):
                    nc.tensor.matmul(psw[:], warm[:, 0:128], warm[:])

                for xg in range(H // XG):
                    psA = psum1.tile([128, XG, OH], mybir.dt.float32, tag="psA")
                    psB = psum1.tile([Z1N, XG, OH], mybir.dt.float32, tag="psB")
                    for xi in range(XG):
                        x = xg * XG + xi
                        ci, xl = chunk_of(x)
                        s = slabs[ci]
                        nc.tensor.matmul(psA[:, xi, :], s[:, xl, 0:128], ay_t[:])
                        nc.tensor.matmul(psB[:, xi, :], s[:, xl, Z1LO:Z], ay_t[:])
                    lo = xg * XG
                    dstA = t2a[:, :, lo:lo + XG]
                    dstB = t2b[:, :, lo:lo + XG]
                    srcA = psA[:].transpose([0, 2, 1])
                    srcB = psB[:].transpose([0, 2, 1])
                    if xg % 2 == 0:
                        nc.scalar.copy(dstA, srcA)
                        nc.vector.tensor_copy(dstB, srcB)
                    else:
                        nc.vector.tensor_copy(dstA, srcA)
                        nc.scalar.copy(dstB, srcB)

            with tc.tile_pool(name="psum2", bufs=3, space="PSUM") as psum2:
                for jb in range(NB):
                    t3 = t3_pool.tile([XPF, JB, Z], bf16, tag="t3")
                    stage = stage_pool.tile([OH, JB, Z], bf16, tag="st")
                    for jg in range(JB // 2):
                        pz = psum2.tile([XPF, 2, RES], mybir.dt.float32, tag="pz")
                        for ji in range(2):
                            j = jb * JB + jg * 2 + ji
                            nc.tensor.matmul(pz[:, ji, 0:C0], t2a[:, j, :], az0_t[:])
                            nc.tensor.matmul(pz[:, ji, C0:RES], t2b[:, j, :], az1_t[:])
                        t3d = t3[:, jg * 2:jg * 2 + 2, :]
                        px = psum2.tile([OH, 2, Z], mybir.dt.float32, tag="px")
                        std = stage[:, jg * 2:jg * 2 + 2, :]
                        if jg % 2 == 0:
                            nc.scalar.copy(t3d, pz[:])
                            nc.tensor.matmul(px[:], ax_t[:], t3d)
                            nc.vector.tensor_copy(std, px[:])
                        else:
                            nc.vector.tensor_copy(t3d, pz[:])
                            nc.tensor.matmul(px[:], ax_t[:], t3d)
                            nc.scalar.copy(std, px[:])
                    j0 = jb * JB
                    if jb == NB - 1:
                        qj = JB // 4
                        for q in range(4):
                            eng = nc.gpsimd if q % 2 == 0 else nc.sync
                            eng.dma_start(
                                out_d[:, j0 + q * qj:j0 + (q + 1) * qj, :],
                                stage[:, q * qj:(q + 1) * qj, :],
                            )
                    elif jb % 2 == 0:
                        nc.gpsimd.dma_start(out_d[:, j0:j0 + JB, :], stage[:])
                    else:
                        nc.sync.dma_start(out_d[:, j0:j0 + JB, :], stage[:])

    nc.compile()
    _CACHE[key] = nc
    return nc


def _in_maps_fb(vol, sub):
    import ml_dtypes

    bf16 = ml_dtypes.bfloat16
    maps = []
    spans = []
    tabs = {}
    for core in range(8):
        b = core >> 2
        ix = (core >> 1) & 1
        iy = core & 1
        if b not in tabs:
            tabs[b] = tuple(_axis_matrix(sub[b, d]) for d in range(3))
        Ax, Ay, Az = tabs[b]
        SX, axs = _band_slice(Ax, ix * OH, OH)
        SY, ays = _band_slice(Ay, iy * OH, OH)
        axp = np.zeros((XPF, OH), dtype=np.float32)
        axp[:H] = axs
        assert not Az[128:, :C0].any(), "az0 band bound violated"
        assert not Az[:Z1LO, C0:].any(), "az1 band bound violated"
        az0 = np.ascontiguousarray(Az[0:128, 0:C0])
        az1 = np.ascontiguousarray(Az[Z1LO:Z, C0:RES])
        slab = np.ascontiguousarray(
            vol[b, SX:SX + H, SY:SY + H, :, 0].transpose(1, 0, 2)
        )
        maps.append({
            "slab": slab.astype(bf16),
            "ax": axp.astype(bf16),
            "ay": ays.astype(bf16),
            "az0": az0.astype(bf16),
            "az1": az1.astype(bf16),
        })
        spans.append((b, ix * OH, iy * OH))
    return maps, spans


def _kernel_fallback(vol, sub):
    global LAST_RESULTS
    from concourse import bass_utils

    nc = _build_fb()
    maps, spans = _in_maps_fb(vol, sub)
    res = bass_utils.run_bass_kernel_spmd(nc, maps, core_ids=list(range(8)))
    LAST_RESULTS = res
    out = np.empty((2, RES, RES, RES, 1), dtype=np.float32)
    for core, (b, x0, y0) in enumerate(spans):
        out[b, x0:x0 + OH, y0:y0 + OH, :, 0] = np.asarray(
            res.results[core]["out"], dtype=np.float32
        )
    return out
